# revision 46
# baseline (speedup 1.0000x reference)
"""Trainium2 Bass kernel for nn_Attention_56487409877769.

NdLinear-qkv -> 16-head attention -> NdLinear-proj, B=4 N=1024 C=1024 H=16.

Sharding: 8 cores = (batch b, head-group g) with b=core//2, g=core%2.
Each core handles batch b and its 8 heads (qkv channel slice 512g:512g+512).
The proj channel matmul is a partial sum over the core's channel slice; the
host adds the two partials per batch plus a rank-1 bias term (the NdLinear
proj biases commute: out = Wp0 @ O @ Wp1c.T + outer(bp0, Wp1.sum(1)) + bp1).

v4 design notes (on top of v3):
- PSUM-source dtype-converting writes are broken in this stack, but
  SBUF->SBUF converting writes (fp32r -> fp8) work for PE consumers
  (verified by minimal repro).  Every fp8 operand produced on device is
  therefore staged PSUM -> SBUF fp32r -> (engine convert) -> SBUF fp8.
- All weight inputs are fp8e4 host-side (4x less input DMA than v3).
- Phase A unchanged (fp8 DoubleRow, 64 instrs).
- Phase B now runs in fp8 DoubleRow too: x1t is converted to x1t8 right
  after its bias add, chains take 4 j-pair steps instead of 8.  4x fewer
  PE cycles than the fp32r version.
- Phase C: scores and U stay fp32r (q/k/v/E fp32r; converting E would cost
  more vector time than U-DR saves).  The per-head normalize now does ONE
  [1,1024] reciprocal on the U PSUM Z-row into ut_sb before the PE
  transpose, and the per-qtile multiply reads its scalar straight out of
  the transposed PSUM tile (v3 issued 64 single-element reciprocals at
  ~0.6us each).  T.T = Wp0 @ O runs in fp8 DR off o8 (converted per
  head-pair).
- Phase D: out = T @ Wp1c.T kept fp32r (a DR D-phase pushes rel err\n  over the 2e-2 gate: the last matmul has no downstream averaging).
"""

import sys

if "/opt/trn_rl_repo" not in sys.path:
    sys.path.insert(0, "/opt/trn_rl_repo")

import numpy as np

B, N, C, H = 4, 1024, 1024, 16
HD = C // H          # 64
SCALE = HD ** -0.5
P = 128
NT = N // P          # 8 partition tiles of the 1024 axes
HPC = 8              # heads per core
VW = HD + 1          # [v | ones] block width per head

_CACHE = {}

LAST_RESULT = None   # test.py reads exec_time_ns / profile off this


def _tt_matmuls(nc, ptt, o8_sb, wp0t8_sb, tt_sb, pi, cp_engines):
    """T.T[d-block pi] = sum_q O8[q, d-block] x Wp0.T8[q, m] in fp8 DR:
    per mch 4 DR j-pair steps + copy-out, then one fp8 convert of the
    finished [128, 1024] tt row; returned as thunks for interleaving."""
    import concourse.mybir as mybir

    fp32 = mybir.dt.float32
    DR = mybir.MatmulPerfMode.DoubleRow

    thunks = []
    box = {}

    def mk(mch, ju):
        def f():
            if ju == 0:
                box[mch] = ptt.tile([P, 512], fp32, tag="tt", name="ps_tt")
            nc.tensor.matmul(
                box[mch][:],
                o8_sb[:, 2 * ju:2 * ju + 2, 128 * pi:128 * pi + 128],
                wp0t8_sb[:, 2 * ju:2 * ju + 2, 512 * mch:512 * mch + 512],
                start=(ju == 0), stop=(ju == 3),
                perf_mode=DR,
            )
        return f

    def mkcopy(mch):
        def f():
            eng = cp_engines[(pi * 2 + mch) % len(cp_engines)]
            if eng is nc.scalar:
                nc.scalar.copy(
                    tt_sb[:, pi, 512 * mch:512 * mch + 512], box[mch][:])
            else:
                eng.tensor_copy(
                    tt_sb[:, pi, 512 * mch:512 * mch + 512], box[mch][:])
        return f

    for mch in range(2):
        for ju in range(4):
            thunks.append(mk(mch, ju))
        thunks.append(mkcopy(mch))
    return thunks


def _build(reps=1, stop=None, debug=False):
    import concourse.mybir as mybir
    import concourse.tile as tile
    from concourse import bacc

    fp32 = mybir.dt.float32
    fp32r = mybir.dt.float32r
    fp8 = mybir.dt.float8e4
    f16 = mybir.dt.float16
    Exp = mybir.ActivationFunctionType.Exp
    Ident = mybir.ActivationFunctionType.Identity
    DR = mybir.MatmulPerfMode.DoubleRow
    Add = mybir.AluOpType.add

    nc = bacc.Bacc("TRN2", target_bir_lowering=False, debug=False)

    def din(name, shape, dt):
        return nc.dram_tensor(name, shape, dt, kind="ExternalInput").ap()

    x_d = din("x8", [NT + 2, P, C], fp8)
    wq0_d = din("wq0t8", [NT + 2, P, N], fp8)
    wqk1_d = din("wqk1t8", [NT, P, 1024], fp8)
    wv1_d = din("wv1t8", [NT, P, 512], fp8)
    bqk1_d = din("bqk1_t", [P, 8], fp32)
    wp0_d = din("wp0t8", [NT, P, N], fp8)
    wp1_d = din("wp1t_r", [4, P, C], fp32r)
    id65_d = din("id65_f", [P, 65], fp32)
    ones_d = din("ones_r", [P, 8], fp32r)
    out_d = nc.dram_tensor("out16", [NT, P, C], f16, kind="ExternalOutput").ap()
    if debug:
        dbg = {
            "d_x1t8": nc.dram_tensor("d_x1t8", [P, NT, N], fp8,
                                     kind="ExternalOutput").ap(),
            "d_q": nc.dram_tensor("d_q", [P, 4, N], fp32r,
                                  kind="ExternalOutput").ap(),
            "d_kt": nc.dram_tensor("d_kt", [P, 4, N], fp32r,
                                   kind="ExternalOutput").ap(),
            "d_vpad": nc.dram_tensor("d_vpad", [P, NT, HPC * VW], fp32r,
                                     kind="ExternalOutput").ap(),
            "d_o": nc.dram_tensor("d_o", [P, NT, 512], fp32r,
                                  kind="ExternalOutput").ap(),
            "d_tt": nc.dram_tensor("d_tt", [P, 4, N], fp32r,
                                   kind="ExternalOutput").ap(),
            "d_wqk1": nc.dram_tensor("d_wqk1", [P, NT, 1024], fp8,
                                     kind="ExternalOutput").ap(),
            "d_x1t": nc.dram_tensor("d_x1t", [P, NT, N], fp32r,
                                    kind="ExternalOutput").ap(),
        }

    with tile.TileContext(nc) as tc:
      for _rep in range(reps):
        # ---------------- pools (LIFO close order) ---------------------------
        stp_cm = tc.tile_pool(name="stage", bufs=4)
        stp = stp_cm.__enter__()

        qkv_cm = tc.tile_pool(name="qkv", bufs=1)
        qkvp = qkv_cm.__enter__()
        # q/k: [128 part = 2 heads x 64 d, head-pair, 1024 n]
        q_sb = qkvp.tile([P, 4, N], fp32r, name="q_sb")
        kt_sb = qkvp.tile([P, 4, N], fp32r, name="kt_sb")
        vpad_sb = qkvp.tile([P, NT, HPC * VW], fp32r, name="vpad_sb")
        id65_sb = qkvp.tile([P, 65], fp32, name="id65_sb")
        ones_sb = qkvp.tile([P, HPC], fp32r, name="ones_sb")
        # x1t8 + the B weights live here so the last two qk groups can run
        # underneath phase C's exp stream (their pools would otherwise close)
        x1t8_sb = qkvp.tile([P, NT, N], fp8, name="x1t8_sb")
        # fp8 q/k for the DoubleRow score matmuls of heads 2..7: flat fp8
        # copies of q_sb/kt_sb, then DMA-folded into [32, 2, N] per head
        # (packed two heads per tile at base partitions 0/32)
        q8f_sb = qkvp.tile([P, 4, N], fp8, name="q8f_sb")
        k8f_sb = qkvp.tile([P, 4, N], fp8, name="k8f_sb")
        qp8 = {tp: qkvp.tile([64, 2, N], fp8, name=f"qp8_{tp}")
               for tp in (1, 2, 3)}
        kp8 = {tp: qkvp.tile([64, 2, N], fp8, name=f"kp8_{tp}")
               for tp in (1, 2, 3)}
        wqk1t_sb = qkvp.tile([P, NT, 1024], fp8, name="wqk1t_sb")
        wv1t_sb = qkvp.tile([P, NT, 512], fp8, name="wv1t_sb")
        bqk1_sb = qkvp.tile([P, 8], fp32, name="bqk1_sb")

        # ---------------- phase A: x1T = Wq0 @ x[b] (fp8 DoubleRow) -----------
        ab_cm = tc.tile_pool(name="ab", bufs=1)
        abp = ab_cm.__enter__()
        wb_cm = tc.tile_pool(name="wb", bufs=1)
        wbp = wb_cm.__enter__()
        wa_cm = tc.tile_pool(name="wa", bufs=1)
        wap = wa_cm.__enter__()
        psb_cm = tc.tile_pool(name="psb", bufs=4, space="PSUM")
        psb = psb_cm.__enter__()
        psa_cm = tc.tile_pool(name="psa", bufs=4, space="PSUM")
        psa = psa_cm.__enter__()

        x_sb = wap.tile([P, NT + 2, C], fp8, name="x_sb")
        wq0t_sb = wap.tile([P, NT + 2, N], fp8, name="wq0t_sb")
        x1t_sb = abp.tile([P, NT, N], fp32r, name="x1t_sb")

        # tiny constants first so the vpad ones-fill clears the DVE queue
        # long before phase B needs it
        nc.sync.dma_start(out=ones_sb[:], in_=ones_d)
        nc.sync.dma_start(out=id65_sb[:], in_=id65_d)
        for t in range(NT):
            od = vpad_sb[:, t, :].rearrange(
                "p (h j) -> p h j", h=HPC)[:, :, HD:VW]
            nc.vector.tensor_copy(od, ones_sb[:, :, None])
        # interleave x/wq0t pair-DMAs so the chains can start early
        nc.sync.dma_start(out=x_sb[:, 0:2], in_=x_d[0:2])
        nc.gpsimd.dma_start(out=wq0t_sb[:, 0:2], in_=wq0_d[0:2])
        nc.sync.dma_start(out=x_sb[:, 8:10], in_=x_d[8:10])
        nc.gpsimd.dma_start(out=wq0t_sb[:, 8:10], in_=wq0_d[8:10])
        for t in range(1, 4):
            nc.sync.dma_start(out=x_sb[:, 2 * t:2 * t + 2], in_=x_d[2 * t:2 * t + 2])
            nc.gpsimd.dma_start(
                out=wq0t_sb[:, 2 * t:2 * t + 2], in_=wq0_d[2 * t:2 * t + 2])
        # NOTE: tile[:, t:t+2] <- dram[t:t+2] pair-DMAs permute rows within the
        # pair (flat element copy, (s,p,c) source order vs (p,s,c) dest). That
        # is harmless for phase A (both operands go through the same mangle and
        # contractions are invariant under shared row permutations) but fatal
        # when mixed with device-built natural-layout tiles, so weights that
        # multiply x1t8/o8 are DMA'd one tile at a time.
        for t in range(NT):
            nc.sync.dma_start(out=wqk1t_sb[:, t], in_=wqk1_d[t])
        nc.sync.dma_start(out=bqk1_sb[:], in_=bqk1_d)
        for t in range(NT):
            nc.sync.dma_start(out=wv1t_sb[:, t], in_=wv1_d[t])

        # A chains: groups of 4 (2 ct x 2 mch), j-outer DoubleRow over k-pairs.
        # After each group: bias-add to x1t (DVE/Pool split) and fp8 convert
        # to x1t8 (ACT), pipelined so B can start as soon as pairs exist.
        for cg in range(4):
            cts = (2 * cg, 2 * cg + 1)
            ps = {}
            pool = psa if cg % 2 == 0 else psb
            ptag = "psa" if cg % 2 == 0 else "psb"
            for ct in cts:
                for mch in range(2):
                    ps[(ct, mch)] = pool.tile([P, 512], fp32, tag=ptag, name="ps_a")
            # 5th step contracts the host-built [ones | bq0] pair, folding
            # the seq-axis bias into the matmul itself
            for j in range(5):
                for ct in cts:
                    for mch in range(2):
                        nc.tensor.matmul(
                            ps[(ct, mch)][:],
                            x_sb[:, 2 * j:2 * j + 2, 128 * ct:128 * ct + 128],
                            wq0t_sb[:, 2 * j:2 * j + 2, 512 * mch:512 * mch + 512],
                            start=(j == 0), stop=(j == 4),
                            perf_mode=DR,
                        )
            # pass 1: PSUM -> SBUF fp32r (no bias; GPSIMD cannot read PSUM)
            for ct in cts:
                for mch in range(2):
                    msl = slice(512 * mch, 512 * mch + 512)
                    if (ct + mch) % 2 == 0:
                        nc.vector.tensor_copy(x1t_sb[:, ct, msl], ps[(ct, mch)][:])
                    else:
                        nc.scalar.copy(x1t_sb[:, ct, msl], ps[(ct, mch)][:])
            # pass 2: plain fp8 convert (bias already in the matmul),
            # SBUF -> SBUF at [128,512] halves, spread across all three
            # vector engines so no single queue paces the x1t8 chain
            for ct in cts:
                for mch in range(2):
                    msl = slice(512 * mch, 512 * mch + 512)
                    if ct < 4:
                        nc.gpsimd.tensor_copy(
                            x1t8_sb[:, ct, msl], x1t_sb[:, ct, msl])
                    elif ct < 6:
                        nc.scalar.copy(x1t8_sb[:, ct, msl], x1t_sb[:, ct, msl])
                    else:
                        nc.vector.tensor_copy(
                            x1t8_sb[:, ct, msl], x1t_sb[:, ct, msl])
        wa_cm.__exit__(None, None, None)

        # -------- phase B: fp8 DR chains, 4 j-pair steps each -----------------
        def b_group8(dts_or_nts, kind, pool, ptag):
            ps = {}
            if kind == "qk":
                keys = [(dt, mch) for dt in dts_or_nts for mch in range(2)]
            else:
                keys = list(dts_or_nts)
            for key in keys:
                ps[key] = pool.tile([P, 512], fp32, tag=ptag, name="ps_b")
            for ju in range(4):
                jsl = slice(2 * ju, 2 * ju + 2)
                if kind == "qk":
                    for dt in dts_or_nts:
                        for mch in range(2):
                            nc.tensor.matmul(
                                ps[(dt, mch)][:],
                                wqk1t_sb[:, jsl, 128 * dt:128 * dt + 128],
                                x1t8_sb[:, jsl, 512 * mch:512 * mch + 512],
                                start=(ju == 0), stop=(ju == 3),
                                perf_mode=DR,
                            )
                else:
                    for nt in dts_or_nts:
                        nc.tensor.matmul(
                            ps[nt][:],
                            x1t8_sb[:, jsl, 128 * nt:128 * nt + 128],
                            wv1t_sb[:, jsl, :],
                            start=(ju == 0), stop=(ju == 3),
                            perf_mode=DR,
                        )

            def copies(engs):
                if kind == "qk":
                    i = 0
                    for dt in dts_or_nts:
                        for mch in range(2):
                            msl = slice(512 * mch, 512 * mch + 512)
                            dst = q_sb if dt < 4 else kt_sb
                            eng = engs[i % len(engs)]
                            i += 1
                            if eng is nc.scalar:
                                nc.scalar.activation(
                                    dst[:, dt % 4, msl], ps[(dt, mch)][:],
                                    Ident, bias=bqk1_sb[:, dt:dt + 1])
                            else:
                                eng.tensor_scalar_add(
                                    dst[:, dt % 4, msl], ps[(dt, mch)][:],
                                    bqk1_sb[:, dt:dt + 1])
                    return
                # v bias is folded into the host-side rank-1 output bias
                # (softmax weights sum to 1 so O just shifts by bv), leaving
                # a plain PSUM -> SBUF copy here.
                i = 0
                for nt in dts_or_nts:
                    vdst = vpad_sb[:, nt, :].rearrange(
                        "p (h j) -> p h j", h=HPC)[:, :, 0:HD]
                    vsrc = ps[nt][:].rearrange("p (h j) -> p h j", h=HPC)
                    eng = engs[i % len(engs)]
                    i += 1
                    if eng is nc.scalar:
                        nc.scalar.copy(vdst, vsrc)
                    else:
                        eng.tensor_copy(vdst, vsrc)

            return copies

        # only the head-0/1-critical slice of B runs before C: q/k for the
        # first head pair.  Everything else (all v tiles and qk pairs 1/2/3)
        # runs as finish-queue thunks underneath phase C's first two heads,
        # where the exp stream leaves the PE mostly idle.
        cp04 = b_group8((0, 4), "qk", psa, "psa")
        cp04([nc.vector, nc.scalar])
        psa_cm.__exit__(None, None, None)
        psb_cm.__exit__(None, None, None)
        wb_cm.__exit__(None, None, None)
        if debug:
            nc.sync.dma_start(out=dbg["d_x1t8"], in_=x1t8_sb[:])
            nc.sync.dma_start(out=dbg["d_wqk1"], in_=wqk1t_sb[:])
            nc.sync.dma_start(out=dbg["d_x1t"], in_=x1t_sb[:])
            nc.sync.dma_start(out=dbg["d_q"], in_=q_sb[:])
            nc.sync.dma_start(out=dbg["d_kt"], in_=kt_sb[:])
            nc.sync.dma_start(out=dbg["d_vpad"], in_=vpad_sb[:])
        ab_cm.__exit__(None, None, None)
        if stop == "b":
            qkv_cm.__exit__(None, None, None)
            stp_cm.__exit__(None, None, None)
            continue

        # ------------- phase C: attention, pipelined per head -----------------
        otp_cm = tc.tile_pool(name="otp", bufs=1)
        otp = otp_cm.__enter__()
        o_sb = otp.tile([P, NT, 512], fp32r, name="o_sb")
        o8_sb = otp.tile([P, NT, 512], fp8, name="o8_sb")
        tt_sb = otp.tile([P, 4, N], fp32r, name="tt_sb")
        ut_sb = otp.tile([P, N], fp32, name="ut_sb")

        wd_cm = tc.tile_pool(name="wd", bufs=1)
        wdp = wd_cm.__enter__()
        wp0t8_sb = wdp.tile([P, NT, N], fp8, name="wp0t8_sb")
        wp1_sb = wdp.tile([P, 4, C], fp32r, name="wp1_sb")
        for t in range(NT):
            nc.sync.dma_start(out=wp0t8_sb[:, t], in_=wp0_d[t])
        for kd in range(4):
            nc.sync.dma_start(out=wp1_sb[:, kd], in_=wp1_d[kd])

        ep_cm = tc.tile_pool(name="ep", bufs=12)
        ep = ep_cm.__enter__()
        # pst opened LAST so it can close FIRST at the tail (LIFO), freeing
        # its 4 banks for deeper-buffered tail pools
        psu_cm = tc.tile_pool(name="psu", bufs=1, space="PSUM")
        ptt_cm = tc.tile_pool(name="ptt", bufs=2, space="PSUM")
        pst_cm = tc.tile_pool(name="pst", bufs=2, space="PSUM")
        psu = psu_cm.__enter__()
        ptt = ptt_cm.__enter__()
        ptr = ptt   # transposes share the [P,512] ring (first 65 cols used)
        pst = pst_cm.__enter__()

        def u_steps(state, j):
            # U chain steps for key-tiles (2j, 2j+1) of the previous head:
            # ups[65, 1024] += [v|ones].T @ E   (rows 0..63 = U.T, row 64 = Z)
            h, et, ups = state
            vsl = slice(VW * h, VW * h + VW)
            for k in (2 * j, 2 * j + 1):
                for nch in range(2):
                    nc.tensor.matmul(
                        ups[0:VW, 512 * nch:512 * nch + 512],
                        vpad_sb[:, k, vsl],
                        et[k][:, 512 * nch:512 * nch + 512],
                        start=(k == 0), stop=(k == NT - 1),
                    )

        def u_steps4(state, jg):
            # same U chain in 4-ktile groups (used for the second-to-last
            # head so its finish work can drain during the last head)
            h, et, ups = state
            vsl = slice(VW * h, VW * h + VW)
            for k in range(4 * jg, 4 * jg + 4):
                for nch in range(2):
                    nc.tensor.matmul(
                        ups[0:VW, 512 * nch:512 * nch + 512],
                        vpad_sb[:, k, vsl],
                        et[k][:, 512 * nch:512 * nch + 512],
                        start=(k == 0), stop=(k == NT - 1),
                    )

        def o8_convert(h, eng):
            # head h fully normalized -> fp8 half-column copy for the DR proj
            eng.tensor_copy(
                o8_sb[:, :, 64 * h:64 * h + 64],
                o_sb[:, :, 64 * h:64 * h + 64])

        finq = []   # ordered finish-work queue: drained a few thunks per
                    # score step so no burst ever stalls the PE/exp stream
        bq = []     # deferred phase-B chains: drained at a decaying rate so
                    # the late qk pairs land on heads 2..3 where DVE has slack

        def drain(k, kb=0):
            for _ in range(min(k, len(finq))):
                finq.pop(0)()
            for _ in range(min(kb, len(bq))):
                bq.pop(0)()

        def push_b_chain(kind, idx, mch=0):
            # one trailing B chain (4 DR steps + copy-out) through the ptt
            # PSUM ring; T.T work does not need that ring before head 2
            box = {}

            def mk(ju):
                def f():
                    if ju == 0:
                        box[0] = ptt.tile([P, 512], fp32, tag="tt",
                                          name="ps_bq")
                    if kind == "qk":
                        nc.tensor.matmul(
                            box[0][:],
                            wqk1t_sb[:, 2 * ju:2 * ju + 2,
                                     128 * idx:128 * idx + 128],
                            x1t8_sb[:, 2 * ju:2 * ju + 2,
                                    512 * mch:512 * mch + 512],
                            start=(ju == 0), stop=(ju == 3),
                            perf_mode=DR,
                        )
                    else:
                        nc.tensor.matmul(
                            box[0][:],
                            x1t8_sb[:, 2 * ju:2 * ju + 2,
                                    128 * idx:128 * idx + 128],
                            wv1t_sb[:, 2 * ju:2 * ju + 2, :],
                            start=(ju == 0), stop=(ju == 3),
                            perf_mode=DR,
                        )
                return f

            def cp():
                if kind == "qk":
                    msl = slice(512 * mch, 512 * mch + 512)
                    dst = q_sb if idx < 4 else kt_sb
                    nc.vector.tensor_scalar_add(
                        dst[:, idx % 4, msl], box[0][:], bqk1_sb[:, idx:idx + 1])
                else:
                    vdst = vpad_sb[:, idx, :].rearrange(
                        "p (h j) -> p h j", h=HPC)[:, :, 0:HD]
                    vsrc = box[0][:].rearrange("p (h j) -> p h j", h=HPC)
                    nc.vector.tensor_copy(vdst, vsrc)

            for ju in range(4):
                bq.append(mk(ju))
            bq.append(cp)

        def push_qkdr_prep(tp):
            # convert this head-pair's q/k to fp8 and DMA-fold each head's
            # 64 d-rows into [32, 2, N] for the DoubleRow score matmuls
            def cq():
                nc.vector.tensor_copy(q8f_sb[:, tp, :], q_sb[:, tp, :])

            def ck():
                nc.vector.tensor_copy(k8f_sb[:, tp, :], kt_sb[:, tp, :])

            bq.append(cq)
            bq.append(ck)
            for a in range(2):
                for s in range(2):
                    def fold(a=a, s=s):
                        r0 = 64 * a + 32 * s
                        nc.sync.dma_start(
                            out=qp8[tp][32 * a:32 * a + 32, s],
                            in_=q8f_sb[r0:r0 + 32, tp, :])
                        nc.sync.dma_start(
                            out=kp8[tp][32 * a:32 * a + 32, s],
                            in_=k8f_sb[r0:r0 + 32, tp, :])
                    bq.append(fold)

        # order: all v first (U(0) consumes them through head 1), then the
        # qk pairs, each followed by its fp8 score prep
        for nt in range(NT):
            push_b_chain("v", nt)
        for dt, prep in (((1, 5), 1), ((2, 6), 2), ((3, 7), 3)):
            for d in dt:
                for mch in range(2):
                    push_b_chain("qk", d, mch)
            push_qkdr_prep(prep)

        def push_fin(state, trp, boxp, tail=False):
            # finish the previous head's U: rows 0..63 PSUM -> SBUF, Z row
            # reciprocal'd on the way out ([1,1024], one DVE op); then per
            # qtile a PE transpose puts [q, (U.T | 1/Z)] on query partitions
            # and the normalize multiply reads its scalar straight from PSUM.
            # Then the head's o8 half-convert; on odd heads the pair's T.T
            # DR chains follow.  At the tail ACT (exp done) helps out and the
            # PSUM rings are deeper, so nothing ping-pongs single-buffered.
            h, et, ups = state

            def cp():
                (nc.scalar.copy if tail else nc.vector.tensor_copy)(
                    ut_sb[0:HD, :], ups[0:HD, :])

            def rc():
                nc.vector.reciprocal(ut_sb[HD:VW, :], ups[HD:VW, :])

            finq.append(cp)
            finq.append(rc)
            for qt in range(NT):
                def tm(qt=qt, h=h):
                    tr = trp.tile([P, 512], fp32,
                                  tag="tr" if tail else "tt", name="ps_tr")
                    nc.tensor.matmul(
                        tr[:, 0:VW], ut_sb[0:VW, 128 * qt:128 * qt + 128],
                        id65_sb[0:VW, :],
                        start=True, stop=True, is_transpose=True,
                    )
                    nc.vector.tensor_scalar_mul(
                        o_sb[:, qt, 64 * h:64 * h + 64], tr[:, 0:HD],
                        tr[:, HD:VW])
                finq.append(tm)
            oeng = nc.vector if tail else nc.gpsimd
            finq.append(lambda h=h, e=oeng: o8_convert(h, e))
            if h % 2 == 1:
                cp_engs = [nc.vector, nc.scalar] if tail else [nc.vector]
                finq.extend(
                    _tt_matmuls(nc, boxp, o8_sb, wp0t8_sb, tt_sb, h // 2,
                                cp_engs))

        prev = None
        for h in range(HPC):
            tp, a = h // 2, h % 2
            psl = slice(64 * a, 64 * a + 64)
            et = [None] * NT
            last = h == HPC - 1
            for j in range(4):
                for i in range(2):
                    mt = 2 * j + i
                    if h == 0 and j == 0:
                        drain(0, 3)
                    elif h == 0:
                        drain(0, 7)
                    elif h == 1:
                        drain(3, 4)
                    elif h < 4:
                        drain(2, 3)
                    else:
                        drain(2 if not last else 3, 2)
                    # [128,1024] score tiles, bufs=2: the next tile's scores
                    # run while ACT exps the previous one; scores go FIRST in
                    # each step so the exp stream is never behind a PE burst
                    ps = pst.tile([P, 1024], fp32, tag="st", name="ps_st")
                    for mch in range(2):
                        if h < 2:
                            nc.tensor.matmul(
                                ps[:, 512 * mch:512 * mch + 512],
                                kt_sb[psl, tp, 128 * mt:128 * mt + 128],
                                q_sb[psl, tp, 512 * mch:512 * mch + 512],
                                start=True, stop=True,
                            )
                        else:
                            bp = slice(32 * a, 32 * a + 32)
                            nc.tensor.matmul(
                                ps[:, 512 * mch:512 * mch + 512],
                                kp8[tp][bp, :, 128 * mt:128 * mt + 128],
                                qp8[tp][bp, :, 512 * mch:512 * mch + 512],
                                start=True, stop=True,
                                perf_mode=DR,
                            )
                    etj = ep.tile([P, 1024], fp32r, tag="e", name="e_sb")
                    nc.scalar.activation(etj[:], ps[:], Exp, scale=SCALE)
                    et[mt] = etj
                if prev is not None:
                    # for the last head, run the previous head's U chain in
                    # the first two steps and queue its finish work at j==2
                    # so it drains before the tail
                    if not last:
                        u_steps(prev, j)
                    elif j < 2:
                        u_steps4(prev, j)
                    elif j == 2:
                        push_fin(prev, ptr, ptt)
            if prev is not None and not last:
                push_fin(prev, ptr, ptt)   # fin(h-1) drains during head h+1
            ups = psu.tile([P, 1024], fp32, tag="u", name="ps_u")
            prev = (h, et, ups)
        # tail: last head's U chain, drain queue, then its finish work with
        # ACT helping and deeper PSUM rings carved from the freed score pool
        for j in range(4):
            u_steps(prev, j)
            drain(4, len(bq))
        drain(len(finq), len(bq))
        pst_cm.__exit__(None, None, None)
        ptt_cm.__exit__(None, None, None)
        tailp_cm = tc.tile_pool(name="tailp", bufs=2, space="PSUM")
        tailp = tailp_cm.__enter__()
        push_fin(prev, tailp, tailp, tail=True)
        drain(len(finq), len(bq))
        tailp_cm.__exit__(None, None, None)
        psu_cm.__exit__(None, None, None)
        if debug:
            nc.sync.dma_start(out=dbg["d_o"], in_=o_sb[:])
            nc.sync.dma_start(out=dbg["d_tt"], in_=tt_sb[:])
        if stop == "c":
            ep_cm.__exit__(None, None, None)
            wd_cm.__exit__(None, None, None)
            otp_cm.__exit__(None, None, None)
            qkv_cm.__exit__(None, None, None)
            stp_cm.__exit__(None, None, None)
            continue

        # ---------- phase D: out = T @ Wp1c.T (fp8 DR) ------------------------
        psd_cm = tc.tile_pool(name="psd", bufs=4, space="PSUM")
        psd = psd_cm.__enter__()
        d_engs = [nc.vector, nc.scalar]
        for mt in range(NT):
            for dch in range(2):
                dsl = slice(512 * dch, 512 * dch + 512)
                ps = psd.tile([P, 512], fp32, tag="psd", name="ps_o")
                for kd in range(4):
                    nc.tensor.matmul(
                        ps[:],
                        tt_sb[:, kd, 128 * mt:128 * mt + 128],
                        wp1_sb[:, kd, dsl],
                        start=(kd == 0), stop=(kd == 3),
                    )
                ostage = stp.tile([P, 512], f16, tag="ost", name="out_stage")
                eng = d_engs[(2 * mt + dch) % 2]
                if eng is nc.scalar:
                    nc.scalar.copy(ostage[:], ps[:])
                else:
                    eng.tensor_copy(ostage[:], ps[:])
                nc.sync.dma_start(out=out_d[mt, :, dsl], in_=ostage[:])
        psd_cm.__exit__(None, None, None)
        ep_cm.__exit__(None, None, None)
        wd_cm.__exit__(None, None, None)
        otp_cm.__exit__(None, None, None)
        qkv_cm.__exit__(None, None, None)
        stp_cm.__exit__(None, None, None)

    nc.compile()
    return nc


def _get_nc(reps=1):
    key = ("nc", reps)
    if key not in _CACHE:
        _CACHE[key] = _build(reps)
    return _CACHE[key]


def _in_maps(x, Wq0, bq0, Wq1, bq1, Wp0, bp0, Wp1, bp1):
    import ml_dtypes

    f = np.float32
    e4 = ml_dtypes.float8_e4m3
    x = np.asarray(x, f)
    Wq0 = np.asarray(Wq0, f); bq0 = np.asarray(bq0, f)
    Wq1 = np.asarray(Wq1, f); bq1 = np.asarray(bq1, f)
    Wp0 = np.asarray(Wp0, f); Wp1 = np.asarray(Wp1, f)
    wq0t8 = np.zeros((NT + 2, P, N), e4)
    wq0t8[:NT] = Wq0.T.reshape(NT, P, N).astype(e4)
    wq0t8[NT, 0, :] = bq0.astype(e4)          # bias row (contraction n=1024)
    wp0t8 = np.ascontiguousarray(Wp0.T.reshape(NT, P, N)).astype(e4)
    id65 = np.zeros((P, 65), f)
    id65[:65, :] = np.eye(65, dtype=f)
    x8aug_by_b = {}
    for b in range(B):
        xa = np.zeros((NT + 2, P, C), e4)
        xa[:NT] = x[b].reshape(NT, P, C).astype(e4)
        xa[NT, 0, :] = e4(1.0)                # ones row pairs with the bias row
        x8aug_by_b[b] = xa
    maps = []
    for core in range(8):
        b, g = core // 2, core % 2
        x8aug = x8aug_by_b[b]
        # natural layout: qk tile dt<4 = q head-pair (2dt, 2dt+1), dt>=4 = k
        perm = np.concatenate([
            np.arange(512 * g, 512 * g + 512),
            np.arange(C + 512 * g, C + 512 * g + 512)])
        wqk1 = Wq1[perm]                                      # (1024 d', 1024 c)
        vs = slice(2 * C + 512 * g, 2 * C + 512 * g + 512)
        wp1t = np.ascontiguousarray(Wp1[:, 512 * g:512 * g + 512].T.reshape(4, P, C))
        m = {
            "x8": x8aug,
            "wq0t8": wq0t8,
            "wqk1t8": np.ascontiguousarray(wqk1.T.reshape(NT, P, 1024)).astype(e4),
            "wv1t8": np.ascontiguousarray(Wq1[vs].T.reshape(NT, P, 512)).astype(e4),
            "bqk1_t": np.ascontiguousarray(bq1[perm].reshape(8, P).T),
            "wp0t8": wp0t8,
            "wp1t_r": wp1t,
            "id65_f": id65,
            "ones_r": np.ones((P, 8), f),
        }
        maps.append(m)
    return maps


def kernel(x, Wq0, bq0, Wq1, bq1, Wp0, bp0, Wp1, bp1):
    global LAST_RESULT
    import os

    # The SPMD execute path needs jax's axon PJRT backend; a harness that
    # pinned JAX_PLATFORMS=cpu (common for running the jax reference) would
    # otherwise hide the NeuronCores from this process.
    if "axon" not in os.environ.get("JAX_PLATFORMS", "axon"):
        os.environ.pop("JAX_PLATFORMS", None)
    # This container lacks antenv.axon_hooks, so the BASS_TRACE=1 NTFF path
    # in run_bass_kernel_spmd raises ModuleNotFoundError. Force tracing off
    # (a crash would otherwise replace a working run).
    os.environ["BASS_NEVER_TRACE"] = "1"
    from concourse.bass_utils import run_bass_kernel_spmd

    nc = _get_nc()
    maps = _in_maps(x, Wq0, bq0, Wq1, bq1, Wp0, bp0, Wp1, bp1)
    res = run_bass_kernel_spmd(nc, maps, list(range(8)))
    LAST_RESULT = res
    parts = [np.asarray(r["out16"], np.float32).reshape(N, C)
             for r in res.results]
    f = np.float32
    bp0 = np.asarray(bp0, f); bp1 = np.asarray(bp1, f)
    Wp1 = np.asarray(Wp1, f)
    bq1 = np.asarray(bq1, f); Wp0 = np.asarray(Wp0, f)
    bias = np.outer(bp0, Wp1.sum(axis=1)) + bp1[None, :]
    # v-bias folded out of the device kernel: softmax rows sum to 1, so the
    # attention output shifts by bv and proj maps that to a rank-1 term.
    bias = bias + np.outer(Wp0.sum(axis=1), Wp1 @ bq1[2 * C:3 * C])
    out = np.stack(
        [parts[2 * b] + parts[2 * b + 1] + bias for b in range(B)], 0)
    return out.astype(f)


# revision 47
# speedup vs baseline: 1.0093x; 1.0093x over previous
"""Trainium2 Bass kernel for nn_Attention_56487409877769.

NdLinear-qkv -> 16-head attention -> NdLinear-proj, B=4 N=1024 C=1024 H=16.

Sharding: 8 cores = (batch b, head-group g) with b=core//2, g=core%2.
Each core handles batch b and its 8 heads (qkv channel slice 512g:512g+512).
The proj channel matmul is a partial sum over the core's channel slice; the
host adds the two partials per batch plus a rank-1 bias term (the NdLinear
proj biases commute: out = Wp0 @ O @ Wp1c.T + outer(bp0, Wp1.sum(1)) + bp1).

v4 design notes (on top of v3):
- PSUM-source dtype-converting writes are broken in this stack, but
  SBUF->SBUF converting writes (fp32r -> fp8) work for PE consumers
  (verified by minimal repro).  Every fp8 operand produced on device is
  therefore staged PSUM -> SBUF fp32r -> (engine convert) -> SBUF fp8.
- All weight inputs are fp8e4 host-side (4x less input DMA than v3).
- Phase A unchanged (fp8 DoubleRow, 64 instrs).
- Phase B now runs in fp8 DoubleRow too: x1t is converted to x1t8 right
  after its bias add, chains take 4 j-pair steps instead of 8.  4x fewer
  PE cycles than the fp32r version.
- Phase C: scores and U stay fp32r (q/k/v/E fp32r; converting E would cost
  more vector time than U-DR saves).  The per-head normalize now does ONE
  [1,1024] reciprocal on the U PSUM Z-row into ut_sb before the PE
  transpose, and the per-qtile multiply reads its scalar straight out of
  the transposed PSUM tile (v3 issued 64 single-element reciprocals at
  ~0.6us each).  T.T = Wp0 @ O runs in fp8 DR off o8 (converted per
  head-pair).
- Phase D: out = T @ Wp1c.T kept fp32r (a DR D-phase pushes rel err\n  over the 2e-2 gate: the last matmul has no downstream averaging).
"""

import sys

if "/opt/trn_rl_repo" not in sys.path:
    sys.path.insert(0, "/opt/trn_rl_repo")

import numpy as np

B, N, C, H = 4, 1024, 1024, 16
HD = C // H          # 64
SCALE = HD ** -0.5
P = 128
NT = N // P          # 8 partition tiles of the 1024 axes
HPC = 8              # heads per core
VW = HD + 1          # [v | ones] block width per head

_CACHE = {}

LAST_RESULT = None   # test.py reads exec_time_ns / profile off this


def _tt_matmuls(nc, ptt, o8_sb, wp0t8_sb, tt_sb, pi, cp_engines):
    """T.T[d-block pi] = sum_q O8[q, d-block] x Wp0.T8[q, m] in fp8 DR:
    per mch 4 DR j-pair steps + copy-out, then one fp8 convert of the
    finished [128, 1024] tt row; returned as thunks for interleaving."""
    import concourse.mybir as mybir

    fp32 = mybir.dt.float32
    DR = mybir.MatmulPerfMode.DoubleRow

    thunks = []
    box = {}

    def mk(mch, ju):
        def f():
            if ju == 0:
                box[mch] = ptt.tile([P, 512], fp32, tag="tt", name="ps_tt")
            nc.tensor.matmul(
                box[mch][:],
                o8_sb[:, 2 * ju:2 * ju + 2, 128 * pi:128 * pi + 128],
                wp0t8_sb[:, 2 * ju:2 * ju + 2, 512 * mch:512 * mch + 512],
                start=(ju == 0), stop=(ju == 3),
                perf_mode=DR,
            )
        return f

    def mkcopy(mch):
        def f():
            eng = cp_engines[(pi * 2 + mch) % len(cp_engines)]
            if eng is nc.scalar:
                nc.scalar.copy(
                    tt_sb[:, pi, 512 * mch:512 * mch + 512], box[mch][:])
            else:
                eng.tensor_copy(
                    tt_sb[:, pi, 512 * mch:512 * mch + 512], box[mch][:])
        return f

    for mch in range(2):
        for ju in range(4):
            thunks.append(mk(mch, ju))
        thunks.append(mkcopy(mch))
    return thunks


def _build(reps=1, stop=None, debug=False):
    import concourse.mybir as mybir
    import concourse.tile as tile
    from concourse import bacc

    fp32 = mybir.dt.float32
    fp32r = mybir.dt.float32r
    fp8 = mybir.dt.float8e4
    f16 = mybir.dt.float16
    Exp = mybir.ActivationFunctionType.Exp
    Ident = mybir.ActivationFunctionType.Identity
    DR = mybir.MatmulPerfMode.DoubleRow
    Add = mybir.AluOpType.add

    nc = bacc.Bacc("TRN2", target_bir_lowering=False, debug=False)

    def din(name, shape, dt):
        return nc.dram_tensor(name, shape, dt, kind="ExternalInput").ap()

    x_d = din("x8", [NT + 2, P, C], fp8)
    wq0_d = din("wq0t8", [NT + 2, P, N], fp8)
    wqk1_d = din("wqk1t8", [NT, P, 1024], fp8)
    wv1_d = din("wv1t8", [NT, P, 512], fp8)
    bqk1_d = din("bqk1_t", [P, 8], fp32)
    wp0_d = din("wp0t8", [NT, P, N], fp8)
    wp1_d = din("wp1t_r", [4, P, C], fp32r)
    id65_d = din("id65_f", [P, 65], fp32)
    ones_d = din("ones_r", [P, 8], fp32r)
    out_d = nc.dram_tensor("out16", [NT, P, C], f16, kind="ExternalOutput").ap()
    if debug:
        dbg = {
            "d_x1t8": nc.dram_tensor("d_x1t8", [P, NT, N], fp8,
                                     kind="ExternalOutput").ap(),
            "d_q": nc.dram_tensor("d_q", [P, 4, N], fp32r,
                                  kind="ExternalOutput").ap(),
            "d_kt": nc.dram_tensor("d_kt", [P, 4, N], fp32r,
                                   kind="ExternalOutput").ap(),
            "d_vpad": nc.dram_tensor("d_vpad", [P, NT, HPC * VW], fp32r,
                                     kind="ExternalOutput").ap(),
            "d_o": nc.dram_tensor("d_o", [P, NT, 512], fp32r,
                                  kind="ExternalOutput").ap(),
            "d_tt": nc.dram_tensor("d_tt", [P, 4, N], fp32r,
                                   kind="ExternalOutput").ap(),
            "d_wqk1": nc.dram_tensor("d_wqk1", [P, NT, 1024], fp8,
                                     kind="ExternalOutput").ap(),
            "d_x1t": nc.dram_tensor("d_x1t", [P, NT, N], fp32r,
                                    kind="ExternalOutput").ap(),
        }

    with tile.TileContext(nc) as tc:
      for _rep in range(reps):
        # ---------------- pools (LIFO close order) ---------------------------
        stp_cm = tc.tile_pool(name="stage", bufs=4)
        stp = stp_cm.__enter__()

        qkv_cm = tc.tile_pool(name="qkv", bufs=1)
        qkvp = qkv_cm.__enter__()
        # q/k: [128 part = 2 heads x 64 d, head-pair, 1024 n]
        q_sb = qkvp.tile([P, 4, N], fp32r, name="q_sb")
        kt_sb = qkvp.tile([P, 4, N], fp32r, name="kt_sb")
        vpad_sb = qkvp.tile([P, NT, HPC * VW], fp32r, name="vpad_sb")
        id65_sb = qkvp.tile([P, 65], fp32, name="id65_sb")
        ones_sb = qkvp.tile([P, HPC], fp32r, name="ones_sb")
        # x1t8 + the B weights live here so the last two qk groups can run
        # underneath phase C's exp stream (their pools would otherwise close)
        x1t8_sb = qkvp.tile([P, NT, N], fp8, name="x1t8_sb")
        # fp8 q/k for the DoubleRow score matmuls of heads 2..7: flat fp8
        # copies of q_sb/kt_sb, then DMA-folded into [32, 2, N] per head
        # (packed two heads per tile at base partitions 0/32)
        q8f_sb = qkvp.tile([P, 4, N], fp8, name="q8f_sb")
        k8f_sb = qkvp.tile([P, 4, N], fp8, name="k8f_sb")
        qp8 = {tp: qkvp.tile([64, 2, N], fp8, name=f"qp8_{tp}")
               for tp in (1, 2, 3)}
        kp8 = {tp: qkvp.tile([64, 2, N], fp8, name=f"kp8_{tp}")
               for tp in (1, 2, 3)}
        wqk1t_sb = qkvp.tile([P, NT, 1024], fp8, name="wqk1t_sb")
        wv1t_sb = qkvp.tile([P, NT, 512], fp8, name="wv1t_sb")
        bqk1_sb = qkvp.tile([P, 8], fp32, name="bqk1_sb")

        # ---------------- phase A: x1T = Wq0 @ x[b] (fp8 DoubleRow) -----------
        ab_cm = tc.tile_pool(name="ab", bufs=1)
        abp = ab_cm.__enter__()
        wb_cm = tc.tile_pool(name="wb", bufs=1)
        wbp = wb_cm.__enter__()
        wa_cm = tc.tile_pool(name="wa", bufs=1)
        wap = wa_cm.__enter__()
        psb_cm = tc.tile_pool(name="psb", bufs=4, space="PSUM")
        psb = psb_cm.__enter__()
        psa_cm = tc.tile_pool(name="psa", bufs=4, space="PSUM")
        psa = psa_cm.__enter__()

        x_sb = wap.tile([P, NT + 2, C], fp8, name="x_sb")
        wq0t_sb = wap.tile([P, NT + 2, N], fp8, name="wq0t_sb")
        x1t_sb = abp.tile([P, NT, N], fp32r, name="x1t_sb")

        # tiny constants first so the vpad ones-fill clears the DVE queue
        # long before phase B needs it
        nc.sync.dma_start(out=ones_sb[:], in_=ones_d)
        nc.sync.dma_start(out=id65_sb[:], in_=id65_d)
        for t in range(NT):
            od = vpad_sb[:, t, :].rearrange(
                "p (h j) -> p h j", h=HPC)[:, :, HD:VW]
            nc.vector.tensor_copy(od, ones_sb[:, :, None])
        # interleave x/wq0t pair-DMAs so the chains can start early
        nc.sync.dma_start(out=x_sb[:, 0:2], in_=x_d[0:2])
        nc.gpsimd.dma_start(out=wq0t_sb[:, 0:2], in_=wq0_d[0:2])
        nc.sync.dma_start(out=x_sb[:, 8:10], in_=x_d[8:10])
        nc.gpsimd.dma_start(out=wq0t_sb[:, 8:10], in_=wq0_d[8:10])
        for t in range(1, 4):
            nc.sync.dma_start(out=x_sb[:, 2 * t:2 * t + 2], in_=x_d[2 * t:2 * t + 2])
            nc.gpsimd.dma_start(
                out=wq0t_sb[:, 2 * t:2 * t + 2], in_=wq0_d[2 * t:2 * t + 2])
        # NOTE: tile[:, t:t+2] <- dram[t:t+2] pair-DMAs permute rows within the
        # pair (flat element copy, (s,p,c) source order vs (p,s,c) dest). That
        # is harmless for phase A (both operands go through the same mangle and
        # contractions are invariant under shared row permutations) but fatal
        # when mixed with device-built natural-layout tiles, so weights that
        # multiply x1t8/o8 are DMA'd one tile at a time.
        for t in range(NT):
            nc.sync.dma_start(out=wqk1t_sb[:, t], in_=wqk1_d[t])
        nc.sync.dma_start(out=bqk1_sb[:], in_=bqk1_d)
        for t in range(NT):
            nc.sync.dma_start(out=wv1t_sb[:, t], in_=wv1_d[t])

        # A chains: groups of 4 (2 ct x 2 mch), j-outer DoubleRow over k-pairs.
        # After each group: bias-add to x1t (DVE/Pool split) and fp8 convert
        # to x1t8 (ACT), pipelined so B can start as soon as pairs exist.
        for cg in range(4):
            cts = (2 * cg, 2 * cg + 1)
            ps = {}
            pool = psa if cg % 2 == 0 else psb
            ptag = "psa" if cg % 2 == 0 else "psb"
            for ct in cts:
                for mch in range(2):
                    ps[(ct, mch)] = pool.tile([P, 512], fp32, tag=ptag, name="ps_a")
            # 5th step contracts the host-built [ones | bq0] pair, folding
            # the seq-axis bias into the matmul itself
            for j in range(5):
                for ct in cts:
                    for mch in range(2):
                        nc.tensor.matmul(
                            ps[(ct, mch)][:],
                            x_sb[:, 2 * j:2 * j + 2, 128 * ct:128 * ct + 128],
                            wq0t_sb[:, 2 * j:2 * j + 2, 512 * mch:512 * mch + 512],
                            start=(j == 0), stop=(j == 4),
                            perf_mode=DR,
                        )
            # pass 1: PSUM -> SBUF fp32r (no bias; GPSIMD cannot read PSUM)
            for ct in cts:
                for mch in range(2):
                    msl = slice(512 * mch, 512 * mch + 512)
                    if (ct + mch) % 2 == 0:
                        nc.vector.tensor_copy(x1t_sb[:, ct, msl], ps[(ct, mch)][:])
                    else:
                        nc.scalar.copy(x1t_sb[:, ct, msl], ps[(ct, mch)][:])
            # pass 2: plain fp8 convert (bias already in the matmul),
            # SBUF -> SBUF at [128,512] halves, spread across all three
            # vector engines so no single queue paces the x1t8 chain
            for ct in cts:
                for mch in range(2):
                    msl = slice(512 * mch, 512 * mch + 512)
                    if ct < 4:
                        nc.gpsimd.tensor_copy(
                            x1t8_sb[:, ct, msl], x1t_sb[:, ct, msl])
                    elif ct < 6:
                        nc.scalar.copy(x1t8_sb[:, ct, msl], x1t_sb[:, ct, msl])
                    else:
                        nc.vector.tensor_copy(
                            x1t8_sb[:, ct, msl], x1t_sb[:, ct, msl])
        wa_cm.__exit__(None, None, None)

        # -------- phase B: fp8 DR chains, 4 j-pair steps each -----------------
        def b_group8(dts_or_nts, kind, pool, ptag):
            ps = {}
            if kind == "qk":
                keys = [(dt, mch) for dt in dts_or_nts for mch in range(2)]
            else:
                keys = list(dts_or_nts)
            for key in keys:
                ps[key] = pool.tile([P, 512], fp32, tag=ptag, name="ps_b")
            for ju in range(4):
                jsl = slice(2 * ju, 2 * ju + 2)
                if kind == "qk":
                    for dt in dts_or_nts:
                        for mch in range(2):
                            nc.tensor.matmul(
                                ps[(dt, mch)][:],
                                wqk1t_sb[:, jsl, 128 * dt:128 * dt + 128],
                                x1t8_sb[:, jsl, 512 * mch:512 * mch + 512],
                                start=(ju == 0), stop=(ju == 3),
                                perf_mode=DR,
                            )
                else:
                    for nt in dts_or_nts:
                        nc.tensor.matmul(
                            ps[nt][:],
                            x1t8_sb[:, jsl, 128 * nt:128 * nt + 128],
                            wv1t_sb[:, jsl, :],
                            start=(ju == 0), stop=(ju == 3),
                            perf_mode=DR,
                        )

            def copies(engs):
                if kind == "qk":
                    i = 0
                    for dt in dts_or_nts:
                        for mch in range(2):
                            msl = slice(512 * mch, 512 * mch + 512)
                            dst = q_sb if dt < 4 else kt_sb
                            eng = engs[i % len(engs)]
                            i += 1
                            if eng is nc.scalar:
                                nc.scalar.activation(
                                    dst[:, dt % 4, msl], ps[(dt, mch)][:],
                                    Ident, bias=bqk1_sb[:, dt:dt + 1])
                            else:
                                eng.tensor_scalar_add(
                                    dst[:, dt % 4, msl], ps[(dt, mch)][:],
                                    bqk1_sb[:, dt:dt + 1])
                    return
                # v bias is folded into the host-side rank-1 output bias
                # (softmax weights sum to 1 so O just shifts by bv), leaving
                # a plain PSUM -> SBUF copy here.
                i = 0
                for nt in dts_or_nts:
                    vdst = vpad_sb[:, nt, :].rearrange(
                        "p (h j) -> p h j", h=HPC)[:, :, 0:HD]
                    vsrc = ps[nt][:].rearrange("p (h j) -> p h j", h=HPC)
                    eng = engs[i % len(engs)]
                    i += 1
                    if eng is nc.scalar:
                        nc.scalar.copy(vdst, vsrc)
                    else:
                        eng.tensor_copy(vdst, vsrc)

            return copies

        # only the head-0/1-critical slice of B runs before C: q/k for the
        # first head pair.  Everything else (all v tiles and qk pairs 1/2/3)
        # runs as finish-queue thunks underneath phase C's first two heads,
        # where the exp stream leaves the PE mostly idle.
        cp04 = b_group8((0, 4), "qk", psa, "psa")
        cp04([nc.vector, nc.scalar])
        psa_cm.__exit__(None, None, None)
        psb_cm.__exit__(None, None, None)
        wb_cm.__exit__(None, None, None)
        if debug:
            nc.sync.dma_start(out=dbg["d_x1t8"], in_=x1t8_sb[:])
            nc.sync.dma_start(out=dbg["d_wqk1"], in_=wqk1t_sb[:])
            nc.sync.dma_start(out=dbg["d_x1t"], in_=x1t_sb[:])
            nc.sync.dma_start(out=dbg["d_q"], in_=q_sb[:])
            nc.sync.dma_start(out=dbg["d_kt"], in_=kt_sb[:])
            nc.sync.dma_start(out=dbg["d_vpad"], in_=vpad_sb[:])
        ab_cm.__exit__(None, None, None)
        if stop == "b":
            qkv_cm.__exit__(None, None, None)
            stp_cm.__exit__(None, None, None)
            continue

        # ------------- phase C: attention, pipelined per head -----------------
        otp_cm = tc.tile_pool(name="otp", bufs=1)
        otp = otp_cm.__enter__()
        o_sb = otp.tile([P, NT, 512], fp32r, name="o_sb")
        o8_sb = otp.tile([P, NT, 512], fp8, name="o8_sb")
        tt_sb = otp.tile([P, 4, N], fp32r, name="tt_sb")
        ut_sb = otp.tile([P, N], fp32, name="ut_sb")

        wd_cm = tc.tile_pool(name="wd", bufs=1)
        wdp = wd_cm.__enter__()
        wp0t8_sb = wdp.tile([P, NT, N], fp8, name="wp0t8_sb")
        wp1_sb = wdp.tile([P, 4, C], fp32r, name="wp1_sb")
        for t in range(NT):
            nc.sync.dma_start(out=wp0t8_sb[:, t], in_=wp0_d[t])
        for kd in range(4):
            nc.sync.dma_start(out=wp1_sb[:, kd], in_=wp1_d[kd])

        ep_cm = tc.tile_pool(name="ep", bufs=12)
        ep = ep_cm.__enter__()
        # pst opened LAST so it can close FIRST at the tail (LIFO), freeing
        # its 4 banks for deeper-buffered tail pools
        psu_cm = tc.tile_pool(name="psu", bufs=1, space="PSUM")
        ptt_cm = tc.tile_pool(name="ptt", bufs=2, space="PSUM")
        pst_cm = tc.tile_pool(name="pst", bufs=2, space="PSUM")
        psu = psu_cm.__enter__()
        ptt = ptt_cm.__enter__()
        ptr = ptt   # transposes share the [P,512] ring (first 65 cols used)
        pst = pst_cm.__enter__()

        def u_steps(state, j):
            # U chain steps for key-tiles (2j, 2j+1) of the previous head:
            # ups[65, 1024] += [v|ones].T @ E   (rows 0..63 = U.T, row 64 = Z)
            h, et, ups = state
            vsl = slice(VW * h, VW * h + VW)
            for k in (2 * j, 2 * j + 1):
                for nch in range(2):
                    nc.tensor.matmul(
                        ups[0:VW, 512 * nch:512 * nch + 512],
                        vpad_sb[:, k, vsl],
                        et[k][:, 512 * nch:512 * nch + 512],
                        start=(k == 0), stop=(k == NT - 1),
                    )

        def u_steps4(state, jg):
            # same U chain in 4-ktile groups (used for the second-to-last
            # head so its finish work can drain during the last head)
            h, et, ups = state
            vsl = slice(VW * h, VW * h + VW)
            for k in range(4 * jg, 4 * jg + 4):
                for nch in range(2):
                    nc.tensor.matmul(
                        ups[0:VW, 512 * nch:512 * nch + 512],
                        vpad_sb[:, k, vsl],
                        et[k][:, 512 * nch:512 * nch + 512],
                        start=(k == 0), stop=(k == NT - 1),
                    )

        def o8_convert(h, eng):
            # head h fully normalized -> fp8 half-column copy for the DR proj
            eng.tensor_copy(
                o8_sb[:, :, 64 * h:64 * h + 64],
                o_sb[:, :, 64 * h:64 * h + 64])

        finq = []   # ordered finish-work queue: drained a few thunks per
                    # score step so no burst ever stalls the PE/exp stream
        bq = []     # deferred phase-B chains: drained at a decaying rate so
                    # the late qk pairs land on heads 2..3 where DVE has slack

        def drain(k, kb=0):
            for _ in range(min(k, len(finq))):
                finq.pop(0)()
            for _ in range(min(kb, len(bq))):
                bq.pop(0)()

        def push_b_chain(kind, idx, mch=0):
            # one trailing B chain (4 DR steps + copy-out) through the ptt
            # PSUM ring; T.T work does not need that ring before head 2
            box = {}

            def mk(ju):
                def f():
                    if ju == 0:
                        box[0] = ptt.tile([P, 512], fp32, tag="tt",
                                          name="ps_bq")
                    if kind == "qk":
                        nc.tensor.matmul(
                            box[0][:],
                            wqk1t_sb[:, 2 * ju:2 * ju + 2,
                                     128 * idx:128 * idx + 128],
                            x1t8_sb[:, 2 * ju:2 * ju + 2,
                                    512 * mch:512 * mch + 512],
                            start=(ju == 0), stop=(ju == 3),
                            perf_mode=DR,
                        )
                    else:
                        nc.tensor.matmul(
                            box[0][:],
                            x1t8_sb[:, 2 * ju:2 * ju + 2,
                                    128 * idx:128 * idx + 128],
                            wv1t_sb[:, 2 * ju:2 * ju + 2, :],
                            start=(ju == 0), stop=(ju == 3),
                            perf_mode=DR,
                        )
                return f

            def cp():
                if kind == "qk":
                    msl = slice(512 * mch, 512 * mch + 512)
                    dst = q_sb if idx < 4 else kt_sb
                    nc.vector.tensor_scalar_add(
                        dst[:, idx % 4, msl], box[0][:], bqk1_sb[:, idx:idx + 1])
                else:
                    vdst = vpad_sb[:, idx, :].rearrange(
                        "p (h j) -> p h j", h=HPC)[:, :, 0:HD]
                    vsrc = box[0][:].rearrange("p (h j) -> p h j", h=HPC)
                    nc.vector.tensor_copy(vdst, vsrc)

            for ju in range(4):
                bq.append(mk(ju))
            bq.append(cp)

        def push_qkdr_prep(tp):
            # convert this head-pair's q/k to fp8 and DMA-fold each head's
            # 64 d-rows into [32, 2, N] for the DoubleRow score matmuls
            def cq():
                nc.vector.tensor_copy(q8f_sb[:, tp, :], q_sb[:, tp, :])

            def ck():
                nc.vector.tensor_copy(k8f_sb[:, tp, :], kt_sb[:, tp, :])

            bq.append(cq)
            bq.append(ck)
            for a in range(2):
                for s in range(2):
                    def fold(a=a, s=s):
                        r0 = 64 * a + 32 * s
                        nc.sync.dma_start(
                            out=qp8[tp][32 * a:32 * a + 32, s],
                            in_=q8f_sb[r0:r0 + 32, tp, :])
                        nc.sync.dma_start(
                            out=kp8[tp][32 * a:32 * a + 32, s],
                            in_=k8f_sb[r0:r0 + 32, tp, :])
                    bq.append(fold)

        # order: all v first (U(0) consumes them through head 1), then the
        # qk pairs, each followed by its fp8 score prep
        for nt in range(NT):
            push_b_chain("v", nt)
        for dt, prep in (((1, 5), 1), ((2, 6), 2), ((3, 7), 3)):
            for d in dt:
                for mch in range(2):
                    push_b_chain("qk", d, mch)
            push_qkdr_prep(prep)

        def push_fin(state, trp, boxp, tail=False):
            # finish the previous head's U: rows 0..63 PSUM -> SBUF, Z row
            # reciprocal'd on the way out ([1,1024], one DVE op); then per
            # qtile a PE transpose puts [q, (U.T | 1/Z)] on query partitions
            # and the normalize multiply reads its scalar straight from PSUM.
            # Then the head's o8 half-convert; on odd heads the pair's T.T
            # DR chains follow.  At the tail ACT (exp done) helps out and the
            # PSUM rings are deeper, so nothing ping-pongs single-buffered.
            h, et, ups = state

            def cp():
                (nc.scalar.copy if tail else nc.vector.tensor_copy)(
                    ut_sb[0:HD, :], ups[0:HD, :])

            def rc():
                nc.vector.reciprocal(ut_sb[HD:VW, :], ups[HD:VW, :])

            finq.append(cp)
            finq.append(rc)
            for qt in range(NT):
                def tm(qt=qt, h=h):
                    tr = trp.tile([P, 512], fp32,
                                  tag="tr" if tail else "tt", name="ps_tr")
                    nc.tensor.matmul(
                        tr[:, 0:VW], ut_sb[0:VW, 128 * qt:128 * qt + 128],
                        id65_sb[0:VW, :],
                        start=True, stop=True, is_transpose=True,
                    )
                    nc.vector.tensor_scalar_mul(
                        o_sb[:, qt, 64 * h:64 * h + 64], tr[:, 0:HD],
                        tr[:, HD:VW])
                finq.append(tm)
            oeng = nc.vector if tail else nc.gpsimd
            finq.append(lambda h=h, e=oeng: o8_convert(h, e))
            if h % 2 == 1:
                cp_engs = [nc.vector, nc.scalar] if tail else [nc.vector]
                finq.extend(
                    _tt_matmuls(nc, boxp, o8_sb, wp0t8_sb, tt_sb, h // 2,
                                cp_engs))

        prev = None
        for h in range(HPC):
            tp, a = h // 2, h % 2
            psl = slice(64 * a, 64 * a + 64)
            et = [None] * NT
            last = h == HPC - 1
            for j in range(4):
                for i in range(2):
                    mt = 2 * j + i
                    if h == 0 and j == 0:
                        drain(0, 4)
                    elif h == 0:
                        drain(0, 8)
                    elif h == 1:
                        drain(3, 5)
                    elif h < 4:
                        drain(2, 4)
                    else:
                        drain(2 if not last else 3, 2)
                    # [128,1024] score tiles, bufs=2: the next tile's scores
                    # run while ACT exps the previous one; scores go FIRST in
                    # each step so the exp stream is never behind a PE burst
                    ps = pst.tile([P, 1024], fp32, tag="st", name="ps_st")
                    for mch in range(2):
                        if h < 2:
                            nc.tensor.matmul(
                                ps[:, 512 * mch:512 * mch + 512],
                                kt_sb[psl, tp, 128 * mt:128 * mt + 128],
                                q_sb[psl, tp, 512 * mch:512 * mch + 512],
                                start=True, stop=True,
                            )
                        else:
                            bp = slice(32 * a, 32 * a + 32)
                            nc.tensor.matmul(
                                ps[:, 512 * mch:512 * mch + 512],
                                kp8[tp][bp, :, 128 * mt:128 * mt + 128],
                                qp8[tp][bp, :, 512 * mch:512 * mch + 512],
                                start=True, stop=True,
                                perf_mode=DR,
                            )
                    etj = ep.tile([P, 1024], fp32r, tag="e", name="e_sb")
                    nc.scalar.activation(etj[:], ps[:], Exp, scale=SCALE)
                    et[mt] = etj
                if prev is not None:
                    # for the last head, run the previous head's U chain in
                    # the first two steps and queue its finish work at j==2
                    # so it drains before the tail
                    if not last:
                        u_steps(prev, j)
                    elif j < 2:
                        u_steps4(prev, j)
                    elif j == 2:
                        push_fin(prev, ptr, ptt)
            if prev is not None and not last:
                push_fin(prev, ptr, ptt)   # fin(h-1) drains during head h+1
            ups = psu.tile([P, 1024], fp32, tag="u", name="ps_u")
            prev = (h, et, ups)
        # tail: last head's U chain, drain queue, then its finish work with
        # ACT helping and deeper PSUM rings carved from the freed score pool
        for j in range(4):
            u_steps(prev, j)
            drain(4, len(bq))
        drain(len(finq), len(bq))
        pst_cm.__exit__(None, None, None)
        ptt_cm.__exit__(None, None, None)
        tailp_cm = tc.tile_pool(name="tailp", bufs=2, space="PSUM")
        tailp = tailp_cm.__enter__()
        push_fin(prev, tailp, tailp, tail=True)
        drain(len(finq), len(bq))
        tailp_cm.__exit__(None, None, None)
        psu_cm.__exit__(None, None, None)
        if debug:
            nc.sync.dma_start(out=dbg["d_o"], in_=o_sb[:])
            nc.sync.dma_start(out=dbg["d_tt"], in_=tt_sb[:])
        if stop == "c":
            ep_cm.__exit__(None, None, None)
            wd_cm.__exit__(None, None, None)
            otp_cm.__exit__(None, None, None)
            qkv_cm.__exit__(None, None, None)
            stp_cm.__exit__(None, None, None)
            continue

        # ---------- phase D: out = T @ Wp1c.T (fp8 DR) ------------------------
        psd_cm = tc.tile_pool(name="psd", bufs=4, space="PSUM")
        psd = psd_cm.__enter__()
        d_engs = [nc.vector, nc.scalar]
        for mt in range(NT):
            for dch in range(2):
                dsl = slice(512 * dch, 512 * dch + 512)
                ps = psd.tile([P, 512], fp32, tag="psd", name="ps_o")
                for kd in range(4):
                    nc.tensor.matmul(
                        ps[:],
                        tt_sb[:, kd, 128 * mt:128 * mt + 128],
                        wp1_sb[:, kd, dsl],
                        start=(kd == 0), stop=(kd == 3),
                    )
                ostage = stp.tile([P, 512], f16, tag="ost", name="out_stage")
                eng = d_engs[(2 * mt + dch) % 2]
                if eng is nc.scalar:
                    nc.scalar.copy(ostage[:], ps[:])
                else:
                    eng.tensor_copy(ostage[:], ps[:])
                nc.sync.dma_start(out=out_d[mt, :, dsl], in_=ostage[:])
        psd_cm.__exit__(None, None, None)
        ep_cm.__exit__(None, None, None)
        wd_cm.__exit__(None, None, None)
        otp_cm.__exit__(None, None, None)
        qkv_cm.__exit__(None, None, None)
        stp_cm.__exit__(None, None, None)

    nc.compile()
    return nc


def _get_nc(reps=1):
    key = ("nc", reps)
    if key not in _CACHE:
        _CACHE[key] = _build(reps)
    return _CACHE[key]


def _in_maps(x, Wq0, bq0, Wq1, bq1, Wp0, bp0, Wp1, bp1):
    import ml_dtypes

    f = np.float32
    e4 = ml_dtypes.float8_e4m3
    x = np.asarray(x, f)
    Wq0 = np.asarray(Wq0, f); bq0 = np.asarray(bq0, f)
    Wq1 = np.asarray(Wq1, f); bq1 = np.asarray(bq1, f)
    Wp0 = np.asarray(Wp0, f); Wp1 = np.asarray(Wp1, f)
    wq0t8 = np.zeros((NT + 2, P, N), e4)
    wq0t8[:NT] = Wq0.T.reshape(NT, P, N).astype(e4)
    wq0t8[NT, 0, :] = bq0.astype(e4)          # bias row (contraction n=1024)
    wp0t8 = np.ascontiguousarray(Wp0.T.reshape(NT, P, N)).astype(e4)
    id65 = np.zeros((P, 65), f)
    id65[:65, :] = np.eye(65, dtype=f)
    x8aug_by_b = {}
    for b in range(B):
        xa = np.zeros((NT + 2, P, C), e4)
        xa[:NT] = x[b].reshape(NT, P, C).astype(e4)
        xa[NT, 0, :] = e4(1.0)                # ones row pairs with the bias row
        x8aug_by_b[b] = xa
    maps = []
    for core in range(8):
        b, g = core // 2, core % 2
        x8aug = x8aug_by_b[b]
        # natural layout: qk tile dt<4 = q head-pair (2dt, 2dt+1), dt>=4 = k
        perm = np.concatenate([
            np.arange(512 * g, 512 * g + 512),
            np.arange(C + 512 * g, C + 512 * g + 512)])
        wqk1 = Wq1[perm]                                      # (1024 d', 1024 c)
        vs = slice(2 * C + 512 * g, 2 * C + 512 * g + 512)
        wp1t = np.ascontiguousarray(Wp1[:, 512 * g:512 * g + 512].T.reshape(4, P, C))
        m = {
            "x8": x8aug,
            "wq0t8": wq0t8,
            "wqk1t8": np.ascontiguousarray(wqk1.T.reshape(NT, P, 1024)).astype(e4),
            "wv1t8": np.ascontiguousarray(Wq1[vs].T.reshape(NT, P, 512)).astype(e4),
            "bqk1_t": np.ascontiguousarray(bq1[perm].reshape(8, P).T),
            "wp0t8": wp0t8,
            "wp1t_r": wp1t,
            "id65_f": id65,
            "ones_r": np.ones((P, 8), f),
        }
        maps.append(m)
    return maps


def kernel(x, Wq0, bq0, Wq1, bq1, Wp0, bp0, Wp1, bp1):
    global LAST_RESULT
    import os

    # The SPMD execute path needs jax's axon PJRT backend; a harness that
    # pinned JAX_PLATFORMS=cpu (common for running the jax reference) would
    # otherwise hide the NeuronCores from this process.
    if "axon" not in os.environ.get("JAX_PLATFORMS", "axon"):
        os.environ.pop("JAX_PLATFORMS", None)
    # This container lacks antenv.axon_hooks, so the BASS_TRACE=1 NTFF path
    # in run_bass_kernel_spmd raises ModuleNotFoundError. Force tracing off
    # (a crash would otherwise replace a working run).
    os.environ["BASS_NEVER_TRACE"] = "1"
    from concourse.bass_utils import run_bass_kernel_spmd

    nc = _get_nc()
    maps = _in_maps(x, Wq0, bq0, Wq1, bq1, Wp0, bp0, Wp1, bp1)
    res = run_bass_kernel_spmd(nc, maps, list(range(8)))
    LAST_RESULT = res
    parts = [np.asarray(r["out16"], np.float32).reshape(N, C)
             for r in res.results]
    f = np.float32
    bp0 = np.asarray(bp0, f); bp1 = np.asarray(bp1, f)
    Wp1 = np.asarray(Wp1, f)
    bq1 = np.asarray(bq1, f); Wp0 = np.asarray(Wp0, f)
    bias = np.outer(bp0, Wp1.sum(axis=1)) + bp1[None, :]
    # v-bias folded out of the device kernel: softmax rows sum to 1, so the
    # attention output shifts by bv and proj maps that to a rank-1 term.
    bias = bias + np.outer(Wp0.sum(axis=1), Wp1 @ bq1[2 * C:3 * C])
    out = np.stack(
        [parts[2 * b] + parts[2 * b + 1] + bias for b in range(B)], 0)
    return out.astype(f)


# revision 48
# speedup vs baseline: 1.0117x; 1.0023x over previous
"""Trainium2 Bass kernel for nn_Attention_56487409877769.

NdLinear-qkv -> 16-head attention -> NdLinear-proj, B=4 N=1024 C=1024 H=16.

Sharding: 8 cores = (batch b, head-group g) with b=core//2, g=core%2.
Each core handles batch b and its 8 heads (qkv channel slice 512g:512g+512).
The proj channel matmul is a partial sum over the core's channel slice; the
host adds the two partials per batch plus a rank-1 bias term (the NdLinear
proj biases commute: out = Wp0 @ O @ Wp1c.T + outer(bp0, Wp1.sum(1)) + bp1).

v4 design notes (on top of v3):
- PSUM-source dtype-converting writes are broken in this stack, but
  SBUF->SBUF converting writes (fp32r -> fp8) work for PE consumers
  (verified by minimal repro).  Every fp8 operand produced on device is
  therefore staged PSUM -> SBUF fp32r -> (engine convert) -> SBUF fp8.
- All weight inputs are fp8e4 host-side (4x less input DMA than v3).
- Phase A unchanged (fp8 DoubleRow, 64 instrs).
- Phase B now runs in fp8 DoubleRow too: x1t is converted to x1t8 right
  after its bias add, chains take 4 j-pair steps instead of 8.  4x fewer
  PE cycles than the fp32r version.
- Phase C: scores and U stay fp32r (q/k/v/E fp32r; converting E would cost
  more vector time than U-DR saves).  The per-head normalize now does ONE
  [1,1024] reciprocal on the U PSUM Z-row into ut_sb before the PE
  transpose, and the per-qtile multiply reads its scalar straight out of
  the transposed PSUM tile (v3 issued 64 single-element reciprocals at
  ~0.6us each).  T.T = Wp0 @ O runs in fp8 DR off o8 (converted per
  head-pair).
- Phase D: out = T @ Wp1c.T kept fp32r (a DR D-phase pushes rel err\n  over the 2e-2 gate: the last matmul has no downstream averaging).
"""

import sys

if "/opt/trn_rl_repo" not in sys.path:
    sys.path.insert(0, "/opt/trn_rl_repo")

import numpy as np

B, N, C, H = 4, 1024, 1024, 16
HD = C // H          # 64
SCALE = HD ** -0.5
P = 128
NT = N // P          # 8 partition tiles of the 1024 axes
HPC = 8              # heads per core
VW = HD + 1          # [v | ones] block width per head

_CACHE = {}

LAST_RESULT = None   # test.py reads exec_time_ns / profile off this


def _tt_matmuls(nc, ptt, o8_sb, wp0t8_sb, tt_sb, pi, cp_engines):
    """T.T[d-block pi] = sum_q O8[q, d-block] x Wp0.T8[q, m] in fp8 DR:
    per mch 4 DR j-pair steps + copy-out, then one fp8 convert of the
    finished [128, 1024] tt row; returned as thunks for interleaving."""
    import concourse.mybir as mybir

    fp32 = mybir.dt.float32
    DR = mybir.MatmulPerfMode.DoubleRow

    thunks = []
    box = {}

    def mk(mch, ju):
        def f():
            if ju == 0:
                box[mch] = ptt.tile([P, 512], fp32, tag="tt", name="ps_tt")
            nc.tensor.matmul(
                box[mch][:],
                o8_sb[:, 2 * ju:2 * ju + 2, 128 * pi:128 * pi + 128],
                wp0t8_sb[:, 2 * ju:2 * ju + 2, 512 * mch:512 * mch + 512],
                start=(ju == 0), stop=(ju == 3),
                perf_mode=DR,
            )
        return f

    def mkcopy(mch):
        def f():
            eng = cp_engines[(pi * 2 + mch) % len(cp_engines)]
            if eng is nc.scalar:
                nc.scalar.copy(
                    tt_sb[:, pi, 512 * mch:512 * mch + 512], box[mch][:])
            else:
                eng.tensor_copy(
                    tt_sb[:, pi, 512 * mch:512 * mch + 512], box[mch][:])
        return f

    for mch in range(2):
        for ju in range(4):
            thunks.append(mk(mch, ju))
        thunks.append(mkcopy(mch))
    return thunks


def _build(reps=1, stop=None, debug=False):
    import concourse.mybir as mybir
    import concourse.tile as tile
    from concourse import bacc

    fp32 = mybir.dt.float32
    fp32r = mybir.dt.float32r
    fp8 = mybir.dt.float8e4
    f16 = mybir.dt.float16
    Exp = mybir.ActivationFunctionType.Exp
    Ident = mybir.ActivationFunctionType.Identity
    DR = mybir.MatmulPerfMode.DoubleRow
    Add = mybir.AluOpType.add

    nc = bacc.Bacc("TRN2", target_bir_lowering=False, debug=False)

    def din(name, shape, dt):
        return nc.dram_tensor(name, shape, dt, kind="ExternalInput").ap()

    x_d = din("x8", [NT + 2, P, C], fp8)
    wq0_d = din("wq0t8", [NT + 2, P, N], fp8)
    wqk1_d = din("wqk1t8", [NT, P, 1024], fp8)
    wv1_d = din("wv1t8", [NT, P, 512], fp8)
    bqk1_d = din("bqk1_t", [P, 8], fp32)
    wp0_d = din("wp0t8", [NT, P, N], fp8)
    wp1_d = din("wp1t_r", [4, P, C], fp32r)
    id65_d = din("id65_f", [P, 65], fp32)
    ones_d = din("ones_r", [P, 8], fp32r)
    out_d = nc.dram_tensor("out16", [NT, P, C], f16, kind="ExternalOutput").ap()
    if debug:
        dbg = {
            "d_x1t8": nc.dram_tensor("d_x1t8", [P, NT, N], fp8,
                                     kind="ExternalOutput").ap(),
            "d_q": nc.dram_tensor("d_q", [P, 4, N], fp32r,
                                  kind="ExternalOutput").ap(),
            "d_kt": nc.dram_tensor("d_kt", [P, 4, N], fp32r,
                                   kind="ExternalOutput").ap(),
            "d_vpad": nc.dram_tensor("d_vpad", [P, NT, HPC * VW], fp32r,
                                     kind="ExternalOutput").ap(),
            "d_o": nc.dram_tensor("d_o", [P, NT, 512], fp32r,
                                  kind="ExternalOutput").ap(),
            "d_tt": nc.dram_tensor("d_tt", [P, 4, N], fp32r,
                                   kind="ExternalOutput").ap(),
            "d_wqk1": nc.dram_tensor("d_wqk1", [P, NT, 1024], fp8,
                                     kind="ExternalOutput").ap(),
            "d_x1t": nc.dram_tensor("d_x1t", [P, NT, N], fp32r,
                                    kind="ExternalOutput").ap(),
        }

    with tile.TileContext(nc) as tc:
      for _rep in range(reps):
        # ---------------- pools (LIFO close order) ---------------------------
        stp_cm = tc.tile_pool(name="stage", bufs=4)
        stp = stp_cm.__enter__()

        qkv_cm = tc.tile_pool(name="qkv", bufs=1)
        qkvp = qkv_cm.__enter__()
        # q/k: [128 part = 2 heads x 64 d, head-pair, 1024 n]
        q_sb = qkvp.tile([P, 4, N], fp32r, name="q_sb")
        kt_sb = qkvp.tile([P, 4, N], fp32r, name="kt_sb")
        vpad_sb = qkvp.tile([P, NT, HPC * VW], fp32r, name="vpad_sb")
        id65_sb = qkvp.tile([P, 65], fp32, name="id65_sb")
        ones_sb = qkvp.tile([P, HPC], fp32r, name="ones_sb")
        # x1t8 + the B weights live here so the last two qk groups can run
        # underneath phase C's exp stream (their pools would otherwise close)
        x1t8_sb = qkvp.tile([P, NT, N], fp8, name="x1t8_sb")
        # fp8 q/k for the DoubleRow score matmuls of heads 2..7: flat fp8
        # copies of q_sb/kt_sb, then DMA-folded into [32, 2, N] per head
        # (packed two heads per tile at base partitions 0/32)
        q8f_sb = qkvp.tile([P, 4, N], fp8, name="q8f_sb")
        k8f_sb = qkvp.tile([P, 4, N], fp8, name="k8f_sb")
        qp8 = {tp: qkvp.tile([64, 2, N], fp8, name=f"qp8_{tp}")
               for tp in (1, 2, 3)}
        kp8 = {tp: qkvp.tile([64, 2, N], fp8, name=f"kp8_{tp}")
               for tp in (1, 2, 3)}
        wqk1t_sb = qkvp.tile([P, NT, 1024], fp8, name="wqk1t_sb")
        wv1t_sb = qkvp.tile([P, NT, 512], fp8, name="wv1t_sb")
        bqk1_sb = qkvp.tile([P, 8], fp32, name="bqk1_sb")

        # ---------------- phase A: x1T = Wq0 @ x[b] (fp8 DoubleRow) -----------
        ab_cm = tc.tile_pool(name="ab", bufs=1)
        abp = ab_cm.__enter__()
        wb_cm = tc.tile_pool(name="wb", bufs=1)
        wbp = wb_cm.__enter__()
        wa_cm = tc.tile_pool(name="wa", bufs=1)
        wap = wa_cm.__enter__()
        psb_cm = tc.tile_pool(name="psb", bufs=4, space="PSUM")
        psb = psb_cm.__enter__()
        psa_cm = tc.tile_pool(name="psa", bufs=4, space="PSUM")
        psa = psa_cm.__enter__()

        x_sb = wap.tile([P, NT + 2, C], fp8, name="x_sb")
        wq0t_sb = wap.tile([P, NT + 2, N], fp8, name="wq0t_sb")
        x1t_sb = abp.tile([P, NT, N], fp32r, name="x1t_sb")

        # tiny constants first so the vpad ones-fill clears the DVE queue
        # long before phase B needs it
        nc.sync.dma_start(out=ones_sb[:], in_=ones_d)
        nc.sync.dma_start(out=id65_sb[:], in_=id65_d)
        for t in range(NT):
            od = vpad_sb[:, t, :].rearrange(
                "p (h j) -> p h j", h=HPC)[:, :, HD:VW]
            nc.vector.tensor_copy(od, ones_sb[:, :, None])
        # interleave x/wq0t pair-DMAs so the chains can start early
        nc.sync.dma_start(out=x_sb[:, 0:2], in_=x_d[0:2])
        nc.gpsimd.dma_start(out=wq0t_sb[:, 0:2], in_=wq0_d[0:2])
        nc.sync.dma_start(out=x_sb[:, 8:10], in_=x_d[8:10])
        nc.gpsimd.dma_start(out=wq0t_sb[:, 8:10], in_=wq0_d[8:10])
        for t in range(1, 4):
            nc.sync.dma_start(out=x_sb[:, 2 * t:2 * t + 2], in_=x_d[2 * t:2 * t + 2])
            nc.gpsimd.dma_start(
                out=wq0t_sb[:, 2 * t:2 * t + 2], in_=wq0_d[2 * t:2 * t + 2])
        # NOTE: tile[:, t:t+2] <- dram[t:t+2] pair-DMAs permute rows within the
        # pair (flat element copy, (s,p,c) source order vs (p,s,c) dest). That
        # is harmless for phase A (both operands go through the same mangle and
        # contractions are invariant under shared row permutations) but fatal
        # when mixed with device-built natural-layout tiles, so weights that
        # multiply x1t8/o8 are DMA'd one tile at a time.
        for t in range(NT):
            nc.sync.dma_start(out=wqk1t_sb[:, t], in_=wqk1_d[t])
        nc.sync.dma_start(out=bqk1_sb[:], in_=bqk1_d)
        for t in range(NT):
            nc.sync.dma_start(out=wv1t_sb[:, t], in_=wv1_d[t])

        # A chains: groups of 4 (2 ct x 2 mch), j-outer DoubleRow over k-pairs.
        # After each group: bias-add to x1t (DVE/Pool split) and fp8 convert
        # to x1t8 (ACT), pipelined so B can start as soon as pairs exist.
        for cg in range(4):
            cts = (2 * cg, 2 * cg + 1)
            ps = {}
            pool = psa if cg % 2 == 0 else psb
            ptag = "psa" if cg % 2 == 0 else "psb"
            for ct in cts:
                for mch in range(2):
                    ps[(ct, mch)] = pool.tile([P, 512], fp32, tag=ptag, name="ps_a")
            # 5th step contracts the host-built [ones | bq0] pair, folding
            # the seq-axis bias into the matmul itself
            for j in range(5):
                for ct in cts:
                    for mch in range(2):
                        nc.tensor.matmul(
                            ps[(ct, mch)][:],
                            x_sb[:, 2 * j:2 * j + 2, 128 * ct:128 * ct + 128],
                            wq0t_sb[:, 2 * j:2 * j + 2, 512 * mch:512 * mch + 512],
                            start=(j == 0), stop=(j == 4),
                            perf_mode=DR,
                        )
            # pass 1: PSUM -> SBUF fp32r (no bias; GPSIMD cannot read PSUM)
            for ct in cts:
                for mch in range(2):
                    msl = slice(512 * mch, 512 * mch + 512)
                    if (ct + mch) % 2 == 0:
                        nc.vector.tensor_copy(x1t_sb[:, ct, msl], ps[(ct, mch)][:])
                    else:
                        nc.scalar.copy(x1t_sb[:, ct, msl], ps[(ct, mch)][:])
            # pass 2: plain fp8 convert (bias already in the matmul),
            # SBUF -> SBUF at [128,512] halves, spread across all three
            # vector engines so no single queue paces the x1t8 chain
            for ct in cts:
                for mch in range(2):
                    msl = slice(512 * mch, 512 * mch + 512)
                    if ct < 4:
                        nc.gpsimd.tensor_copy(
                            x1t8_sb[:, ct, msl], x1t_sb[:, ct, msl])
                    elif ct < 6:
                        nc.scalar.copy(x1t8_sb[:, ct, msl], x1t_sb[:, ct, msl])
                    else:
                        nc.vector.tensor_copy(
                            x1t8_sb[:, ct, msl], x1t_sb[:, ct, msl])
        wa_cm.__exit__(None, None, None)

        # -------- phase B: fp8 DR chains, 4 j-pair steps each -----------------
        def b_group8(dts_or_nts, kind, pool, ptag):
            ps = {}
            if kind == "qk":
                keys = [(dt, mch) for dt in dts_or_nts for mch in range(2)]
            else:
                keys = list(dts_or_nts)
            for key in keys:
                ps[key] = pool.tile([P, 512], fp32, tag=ptag, name="ps_b")
            for ju in range(4):
                jsl = slice(2 * ju, 2 * ju + 2)
                if kind == "qk":
                    for dt in dts_or_nts:
                        for mch in range(2):
                            nc.tensor.matmul(
                                ps[(dt, mch)][:],
                                wqk1t_sb[:, jsl, 128 * dt:128 * dt + 128],
                                x1t8_sb[:, jsl, 512 * mch:512 * mch + 512],
                                start=(ju == 0), stop=(ju == 3),
                                perf_mode=DR,
                            )
                else:
                    for nt in dts_or_nts:
                        nc.tensor.matmul(
                            ps[nt][:],
                            x1t8_sb[:, jsl, 128 * nt:128 * nt + 128],
                            wv1t_sb[:, jsl, :],
                            start=(ju == 0), stop=(ju == 3),
                            perf_mode=DR,
                        )

            def copies(engs):
                if kind == "qk":
                    i = 0
                    for dt in dts_or_nts:
                        for mch in range(2):
                            msl = slice(512 * mch, 512 * mch + 512)
                            dst = q_sb if dt < 4 else kt_sb
                            eng = engs[i % len(engs)]
                            i += 1
                            if eng is nc.scalar:
                                nc.scalar.activation(
                                    dst[:, dt % 4, msl], ps[(dt, mch)][:],
                                    Ident, bias=bqk1_sb[:, dt:dt + 1])
                            else:
                                eng.tensor_scalar_add(
                                    dst[:, dt % 4, msl], ps[(dt, mch)][:],
                                    bqk1_sb[:, dt:dt + 1])
                    return
                # v bias is folded into the host-side rank-1 output bias
                # (softmax weights sum to 1 so O just shifts by bv), leaving
                # a plain PSUM -> SBUF copy here.
                i = 0
                for nt in dts_or_nts:
                    vdst = vpad_sb[:, nt, :].rearrange(
                        "p (h j) -> p h j", h=HPC)[:, :, 0:HD]
                    vsrc = ps[nt][:].rearrange("p (h j) -> p h j", h=HPC)
                    eng = engs[i % len(engs)]
                    i += 1
                    if eng is nc.scalar:
                        nc.scalar.copy(vdst, vsrc)
                    else:
                        eng.tensor_copy(vdst, vsrc)

            return copies

        # only the head-0/1-critical slice of B runs before C: q/k for the
        # first head pair.  Everything else (all v tiles and qk pairs 1/2/3)
        # runs as finish-queue thunks underneath phase C's first two heads,
        # where the exp stream leaves the PE mostly idle.
        cp04 = b_group8((0, 4), "qk", psa, "psa")
        cp04([nc.vector, nc.scalar])
        psa_cm.__exit__(None, None, None)
        psb_cm.__exit__(None, None, None)
        wb_cm.__exit__(None, None, None)
        if debug:
            nc.sync.dma_start(out=dbg["d_x1t8"], in_=x1t8_sb[:])
            nc.sync.dma_start(out=dbg["d_wqk1"], in_=wqk1t_sb[:])
            nc.sync.dma_start(out=dbg["d_x1t"], in_=x1t_sb[:])
            nc.sync.dma_start(out=dbg["d_q"], in_=q_sb[:])
            nc.sync.dma_start(out=dbg["d_kt"], in_=kt_sb[:])
            nc.sync.dma_start(out=dbg["d_vpad"], in_=vpad_sb[:])
        ab_cm.__exit__(None, None, None)
        if stop == "b":
            qkv_cm.__exit__(None, None, None)
            stp_cm.__exit__(None, None, None)
            continue

        # ------------- phase C: attention, pipelined per head -----------------
        otp_cm = tc.tile_pool(name="otp", bufs=1)
        otp = otp_cm.__enter__()
        o_sb = otp.tile([P, NT, 512], fp32r, name="o_sb")
        o8_sb = otp.tile([P, NT, 512], fp8, name="o8_sb")
        tt_sb = otp.tile([P, 4, N], fp32r, name="tt_sb")
        ut_sb = otp.tile([P, N], fp32, name="ut_sb")

        wd_cm = tc.tile_pool(name="wd", bufs=1)
        wdp = wd_cm.__enter__()
        wp0t8_sb = wdp.tile([P, NT, N], fp8, name="wp0t8_sb")
        wp1_sb = wdp.tile([P, 4, C], fp32r, name="wp1_sb")
        for t in range(NT):
            nc.sync.dma_start(out=wp0t8_sb[:, t], in_=wp0_d[t])
        for kd in range(4):
            nc.sync.dma_start(out=wp1_sb[:, kd], in_=wp1_d[kd])

        ep_cm = tc.tile_pool(name="ep", bufs=12)
        ep = ep_cm.__enter__()
        # pst opened LAST so it can close FIRST at the tail (LIFO), freeing
        # its 4 banks for deeper-buffered tail pools
        psu_cm = tc.tile_pool(name="psu", bufs=1, space="PSUM")
        ptt_cm = tc.tile_pool(name="ptt", bufs=2, space="PSUM")
        pst_cm = tc.tile_pool(name="pst", bufs=2, space="PSUM")
        psu = psu_cm.__enter__()
        ptt = ptt_cm.__enter__()
        ptr = ptt   # transposes share the [P,512] ring (first 65 cols used)
        pst = pst_cm.__enter__()

        def u_steps(state, j):
            # U chain steps for key-tiles (2j, 2j+1) of the previous head:
            # ups[65, 1024] += [v|ones].T @ E   (rows 0..63 = U.T, row 64 = Z)
            h, et, ups = state
            vsl = slice(VW * h, VW * h + VW)
            for k in (2 * j, 2 * j + 1):
                for nch in range(2):
                    nc.tensor.matmul(
                        ups[0:VW, 512 * nch:512 * nch + 512],
                        vpad_sb[:, k, vsl],
                        et[k][:, 512 * nch:512 * nch + 512],
                        start=(k == 0), stop=(k == NT - 1),
                    )

        def u_steps4(state, jg):
            # same U chain in 4-ktile groups (used for the second-to-last
            # head so its finish work can drain during the last head)
            h, et, ups = state
            vsl = slice(VW * h, VW * h + VW)
            for k in range(4 * jg, 4 * jg + 4):
                for nch in range(2):
                    nc.tensor.matmul(
                        ups[0:VW, 512 * nch:512 * nch + 512],
                        vpad_sb[:, k, vsl],
                        et[k][:, 512 * nch:512 * nch + 512],
                        start=(k == 0), stop=(k == NT - 1),
                    )

        def o8_convert(h, eng):
            # head h fully normalized -> fp8 half-column copy for the DR proj
            eng.tensor_copy(
                o8_sb[:, :, 64 * h:64 * h + 64],
                o_sb[:, :, 64 * h:64 * h + 64])

        finq = []   # ordered finish-work queue: drained a few thunks per
                    # score step so no burst ever stalls the PE/exp stream
        bq = []     # deferred phase-B chains: drained at a decaying rate so
                    # the late qk pairs land on heads 2..3 where DVE has slack

        def drain(k, kb=0):
            for _ in range(min(k, len(finq))):
                finq.pop(0)()
            for _ in range(min(kb, len(bq))):
                bq.pop(0)()

        def push_b_chain(kind, idx, mch=0):
            # one trailing B chain (4 DR steps + copy-out) through the ptt
            # PSUM ring; T.T work does not need that ring before head 2
            box = {}

            def mk(ju):
                def f():
                    if ju == 0:
                        box[0] = ptt.tile([P, 512], fp32, tag="tt",
                                          name="ps_bq")
                    if kind == "qk":
                        nc.tensor.matmul(
                            box[0][:],
                            wqk1t_sb[:, 2 * ju:2 * ju + 2,
                                     128 * idx:128 * idx + 128],
                            x1t8_sb[:, 2 * ju:2 * ju + 2,
                                    512 * mch:512 * mch + 512],
                            start=(ju == 0), stop=(ju == 3),
                            perf_mode=DR,
                        )
                    else:
                        nc.tensor.matmul(
                            box[0][:],
                            x1t8_sb[:, 2 * ju:2 * ju + 2,
                                    128 * idx:128 * idx + 128],
                            wv1t_sb[:, 2 * ju:2 * ju + 2, :],
                            start=(ju == 0), stop=(ju == 3),
                            perf_mode=DR,
                        )
                return f

            def cp():
                if kind == "qk":
                    msl = slice(512 * mch, 512 * mch + 512)
                    dst = q_sb if idx < 4 else kt_sb
                    nc.vector.tensor_scalar_add(
                        dst[:, idx % 4, msl], box[0][:], bqk1_sb[:, idx:idx + 1])
                else:
                    vdst = vpad_sb[:, idx, :].rearrange(
                        "p (h j) -> p h j", h=HPC)[:, :, 0:HD]
                    vsrc = box[0][:].rearrange("p (h j) -> p h j", h=HPC)
                    nc.vector.tensor_copy(vdst, vsrc)

            for ju in range(4):
                bq.append(mk(ju))
            bq.append(cp)

        def push_qkdr_prep(tp):
            # convert this head-pair's q/k to fp8 and DMA-fold each head's
            # 64 d-rows into [32, 2, N] for the DoubleRow score matmuls
            def cq():
                nc.vector.tensor_copy(q8f_sb[:, tp, :], q_sb[:, tp, :])

            def ck():
                nc.vector.tensor_copy(k8f_sb[:, tp, :], kt_sb[:, tp, :])

            bq.append(cq)
            bq.append(ck)
            for a in range(2):
                for s in range(2):
                    def fold(a=a, s=s):
                        r0 = 64 * a + 32 * s
                        nc.sync.dma_start(
                            out=qp8[tp][32 * a:32 * a + 32, s],
                            in_=q8f_sb[r0:r0 + 32, tp, :])
                        nc.sync.dma_start(
                            out=kp8[tp][32 * a:32 * a + 32, s],
                            in_=k8f_sb[r0:r0 + 32, tp, :])
                    bq.append(fold)

        # order: all v first (U(0) consumes them through head 1), then the
        # qk pairs, each followed by its fp8 score prep
        for nt in range(NT):
            push_b_chain("v", nt)
        for dt, prep in (((1, 5), 1), ((2, 6), 2), ((3, 7), 3)):
            for d in dt:
                for mch in range(2):
                    push_b_chain("qk", d, mch)
            push_qkdr_prep(prep)

        def push_fin(state, trp, boxp, tail=False):
            # finish the previous head's U: rows 0..63 PSUM -> SBUF, Z row
            # reciprocal'd on the way out ([1,1024], one DVE op); then per
            # qtile a PE transpose puts [q, (U.T | 1/Z)] on query partitions
            # and the normalize multiply reads its scalar straight from PSUM.
            # Then the head's o8 half-convert; on odd heads the pair's T.T
            # DR chains follow.  At the tail ACT (exp done) helps out and the
            # PSUM rings are deeper, so nothing ping-pongs single-buffered.
            h, et, ups = state

            def cp():
                (nc.scalar.copy if tail else nc.vector.tensor_copy)(
                    ut_sb[0:HD, :], ups[0:HD, :])

            def rc():
                nc.vector.reciprocal(ut_sb[HD:VW, :], ups[HD:VW, :])

            finq.append(cp)
            finq.append(rc)
            for qt in range(NT):
                def tm(qt=qt, h=h):
                    tr = trp.tile([P, 512], fp32,
                                  tag="tr" if tail else "tt", name="ps_tr")
                    nc.tensor.matmul(
                        tr[:, 0:VW], ut_sb[0:VW, 128 * qt:128 * qt + 128],
                        id65_sb[0:VW, :],
                        start=True, stop=True, is_transpose=True,
                    )
                    nc.vector.tensor_scalar_mul(
                        o_sb[:, qt, 64 * h:64 * h + 64], tr[:, 0:HD],
                        tr[:, HD:VW])
                finq.append(tm)
            oeng = nc.vector if tail else nc.gpsimd
            finq.append(lambda h=h, e=oeng: o8_convert(h, e))
            if h % 2 == 1:
                cp_engs = [nc.vector, nc.scalar] if tail else [nc.vector]
                finq.extend(
                    _tt_matmuls(nc, boxp, o8_sb, wp0t8_sb, tt_sb, h // 2,
                                cp_engs))

        prev = None
        for h in range(HPC):
            tp, a = h // 2, h % 2
            psl = slice(64 * a, 64 * a + 64)
            et = [None] * NT
            last = h == HPC - 1
            for j in range(4):
                for i in range(2):
                    mt = 2 * j + i
                    if h == 0 and j == 0:
                        drain(0, 4)
                    elif h == 0:
                        drain(0, 8)
                    elif h == 1:
                        drain(3, 6)
                    elif h < 4:
                        drain(2, 4)
                    else:
                        drain(2 if not last else 3, 2)
                    # [128,1024] score tiles, bufs=2: the next tile's scores
                    # run while ACT exps the previous one; scores go FIRST in
                    # each step so the exp stream is never behind a PE burst
                    ps = pst.tile([P, 1024], fp32, tag="st", name="ps_st")
                    for mch in range(2):
                        if h < 2:
                            nc.tensor.matmul(
                                ps[:, 512 * mch:512 * mch + 512],
                                kt_sb[psl, tp, 128 * mt:128 * mt + 128],
                                q_sb[psl, tp, 512 * mch:512 * mch + 512],
                                start=True, stop=True,
                            )
                        else:
                            bp = slice(32 * a, 32 * a + 32)
                            nc.tensor.matmul(
                                ps[:, 512 * mch:512 * mch + 512],
                                kp8[tp][bp, :, 128 * mt:128 * mt + 128],
                                qp8[tp][bp, :, 512 * mch:512 * mch + 512],
                                start=True, stop=True,
                                perf_mode=DR,
                            )
                    etj = ep.tile([P, 1024], fp32r, tag="e", name="e_sb")
                    nc.scalar.activation(etj[:], ps[:], Exp, scale=SCALE)
                    et[mt] = etj
                if prev is not None:
                    # for the last head, run the previous head's U chain in
                    # the first two steps and queue its finish work at j==2
                    # so it drains before the tail
                    if not last:
                        u_steps(prev, j)
                    elif j < 2:
                        u_steps4(prev, j)
                    elif j == 2:
                        push_fin(prev, ptr, ptt)
            if prev is not None and not last:
                push_fin(prev, ptr, ptt)   # fin(h-1) drains during head h+1
            ups = psu.tile([P, 1024], fp32, tag="u", name="ps_u")
            prev = (h, et, ups)
        # tail: last head's U chain, drain queue, then its finish work with
        # ACT helping and deeper PSUM rings carved from the freed score pool
        for j in range(4):
            u_steps(prev, j)
            drain(4, len(bq))
        drain(len(finq), len(bq))
        pst_cm.__exit__(None, None, None)
        ptt_cm.__exit__(None, None, None)
        tailp_cm = tc.tile_pool(name="tailp", bufs=2, space="PSUM")
        tailp = tailp_cm.__enter__()
        push_fin(prev, tailp, tailp, tail=True)
        drain(len(finq), len(bq))
        tailp_cm.__exit__(None, None, None)
        psu_cm.__exit__(None, None, None)
        if debug:
            nc.sync.dma_start(out=dbg["d_o"], in_=o_sb[:])
            nc.sync.dma_start(out=dbg["d_tt"], in_=tt_sb[:])
        if stop == "c":
            ep_cm.__exit__(None, None, None)
            wd_cm.__exit__(None, None, None)
            otp_cm.__exit__(None, None, None)
            qkv_cm.__exit__(None, None, None)
            stp_cm.__exit__(None, None, None)
            continue

        # ---------- phase D: out = T @ Wp1c.T (fp8 DR) ------------------------
        psd_cm = tc.tile_pool(name="psd", bufs=4, space="PSUM")
        psd = psd_cm.__enter__()
        d_engs = [nc.vector, nc.scalar]
        for mt in range(NT):
            for dch in range(2):
                dsl = slice(512 * dch, 512 * dch + 512)
                ps = psd.tile([P, 512], fp32, tag="psd", name="ps_o")
                for kd in range(4):
                    nc.tensor.matmul(
                        ps[:],
                        tt_sb[:, kd, 128 * mt:128 * mt + 128],
                        wp1_sb[:, kd, dsl],
                        start=(kd == 0), stop=(kd == 3),
                    )
                ostage = stp.tile([P, 512], f16, tag="ost", name="out_stage")
                eng = d_engs[(2 * mt + dch) % 2]
                if eng is nc.scalar:
                    nc.scalar.copy(ostage[:], ps[:])
                else:
                    eng.tensor_copy(ostage[:], ps[:])
                nc.sync.dma_start(out=out_d[mt, :, dsl], in_=ostage[:])
        psd_cm.__exit__(None, None, None)
        ep_cm.__exit__(None, None, None)
        wd_cm.__exit__(None, None, None)
        otp_cm.__exit__(None, None, None)
        qkv_cm.__exit__(None, None, None)
        stp_cm.__exit__(None, None, None)

    nc.compile()
    return nc


def _get_nc(reps=1):
    key = ("nc", reps)
    if key not in _CACHE:
        _CACHE[key] = _build(reps)
    return _CACHE[key]


def _in_maps(x, Wq0, bq0, Wq1, bq1, Wp0, bp0, Wp1, bp1):
    import ml_dtypes

    f = np.float32
    e4 = ml_dtypes.float8_e4m3
    x = np.asarray(x, f)
    Wq0 = np.asarray(Wq0, f); bq0 = np.asarray(bq0, f)
    Wq1 = np.asarray(Wq1, f); bq1 = np.asarray(bq1, f)
    Wp0 = np.asarray(Wp0, f); Wp1 = np.asarray(Wp1, f)
    wq0t8 = np.zeros((NT + 2, P, N), e4)
    wq0t8[:NT] = Wq0.T.reshape(NT, P, N).astype(e4)
    wq0t8[NT, 0, :] = bq0.astype(e4)          # bias row (contraction n=1024)
    wp0t8 = np.ascontiguousarray(Wp0.T.reshape(NT, P, N)).astype(e4)
    id65 = np.zeros((P, 65), f)
    id65[:65, :] = np.eye(65, dtype=f)
    x8aug_by_b = {}
    for b in range(B):
        xa = np.zeros((NT + 2, P, C), e4)
        xa[:NT] = x[b].reshape(NT, P, C).astype(e4)
        xa[NT, 0, :] = e4(1.0)                # ones row pairs with the bias row
        x8aug_by_b[b] = xa
    maps = []
    for core in range(8):
        b, g = core // 2, core % 2
        x8aug = x8aug_by_b[b]
        # natural layout: qk tile dt<4 = q head-pair (2dt, 2dt+1), dt>=4 = k
        perm = np.concatenate([
            np.arange(512 * g, 512 * g + 512),
            np.arange(C + 512 * g, C + 512 * g + 512)])
        wqk1 = Wq1[perm]                                      # (1024 d', 1024 c)
        vs = slice(2 * C + 512 * g, 2 * C + 512 * g + 512)
        wp1t = np.ascontiguousarray(Wp1[:, 512 * g:512 * g + 512].T.reshape(4, P, C))
        m = {
            "x8": x8aug,
            "wq0t8": wq0t8,
            "wqk1t8": np.ascontiguousarray(wqk1.T.reshape(NT, P, 1024)).astype(e4),
            "wv1t8": np.ascontiguousarray(Wq1[vs].T.reshape(NT, P, 512)).astype(e4),
            "bqk1_t": np.ascontiguousarray(bq1[perm].reshape(8, P).T),
            "wp0t8": wp0t8,
            "wp1t_r": wp1t,
            "id65_f": id65,
            "ones_r": np.ones((P, 8), f),
        }
        maps.append(m)
    return maps


def kernel(x, Wq0, bq0, Wq1, bq1, Wp0, bp0, Wp1, bp1):
    global LAST_RESULT
    import os

    # The SPMD execute path needs jax's axon PJRT backend; a harness that
    # pinned JAX_PLATFORMS=cpu (common for running the jax reference) would
    # otherwise hide the NeuronCores from this process.
    if "axon" not in os.environ.get("JAX_PLATFORMS", "axon"):
        os.environ.pop("JAX_PLATFORMS", None)
    # This container lacks antenv.axon_hooks, so the BASS_TRACE=1 NTFF path
    # in run_bass_kernel_spmd raises ModuleNotFoundError. Force tracing off
    # (a crash would otherwise replace a working run).
    os.environ["BASS_NEVER_TRACE"] = "1"
    from concourse.bass_utils import run_bass_kernel_spmd

    nc = _get_nc()
    maps = _in_maps(x, Wq0, bq0, Wq1, bq1, Wp0, bp0, Wp1, bp1)
    res = run_bass_kernel_spmd(nc, maps, list(range(8)))
    LAST_RESULT = res
    parts = [np.asarray(r["out16"], np.float32).reshape(N, C)
             for r in res.results]
    f = np.float32
    bp0 = np.asarray(bp0, f); bp1 = np.asarray(bp1, f)
    Wp1 = np.asarray(Wp1, f)
    bq1 = np.asarray(bq1, f); Wp0 = np.asarray(Wp0, f)
    bias = np.outer(bp0, Wp1.sum(axis=1)) + bp1[None, :]
    # v-bias folded out of the device kernel: softmax rows sum to 1, so the
    # attention output shifts by bv and proj maps that to a rank-1 term.
    bias = bias + np.outer(Wp0.sum(axis=1), Wp1 @ bq1[2 * C:3 * C])
    out = np.stack(
        [parts[2 * b] + parts[2 * b + 1] + bias for b in range(B)], 0)
    return out.astype(f)
